# revision 1
# baseline (speedup 1.0000x reference)
import sys

if "/opt/trn_rl_repo" not in sys.path:
    sys.path.insert(0, "/opt/trn_rl_repo")

import numpy as np
import ml_dtypes

import concourse.bass as bass
import concourse.bacc as bacc
import concourse.tile as tile
import concourse.mybir as mybir
from concourse import bass_utils

# Problem shapes (nn_ChebConv): x (16, 12288), L (12288, 12288),
# weights (5, 16, 32), bias (32,). out (32, 12288).
#
# Sharding: core d owns V-columns [d*1536, (d+1)*1536).  Host feeds each
# core lt = L^T[:, cols_d] (so the PE streams L with the contraction dim
# on partitions), row-interleaved within 1024-row groups so each SBUF
# partition reads one contiguous 24 KB chunk per bulk DMA.
#
# Recurrence per step k: psum(16,512)x3 accumulates T_{k-1} @ L^T over 96
# vc-tiles (stationary = all-gathered T_{k-1}^T tiles (128,16), moving =
# lt tiles (128,512) bf16); one DVE op forms T_k = 2*psum - T_{k-2}; the
# local (16,1536) chunk is PE-transposed, cast to bf16 and all-gathered
# for the next step's stationary.  The first RES_T vc-tiles of lt stay
# resident in SBUF across all 4 steps (read once instead of 4x).
C_IN = 16
C_OUT = 32
K_CHEB = 5
V = 12288
N_CORES = 8
VLOC = V // N_CORES          # 1536 columns of the V axis per core
P = 128
NT_VC = V // P               # 96 contraction tiles per step
N_CH = VLOC // 512           # 3 psum chunks of 512
NB = 4                       # vc-tiles per bulk lt DMA (512 rows)
NG = NT_VC // NB             # 24 groups
RES_T = 32                   # vc-tiles resident in SBUF (of 96)
RES_G = RES_T // NB          # resident groups
LT_BUFS = 6
NB_S = VLOC // P             # stationary group: 12 vc-tiles = one rank chunk
NG_S = V // (P * NB_S)       # 8 stationary groups

_CACHE: dict = {}


def _build(cfg: str):
    if cfg == "bf16":
        mm_dt = mybir.dt.bfloat16
    elif cfg == "f32r":
        mm_dt = mybir.dt.float32r
    else:
        mm_dt = mybir.dt.float32
    f32 = mybir.dt.float32

    res_g = RES_G if cfg == "bf16" else 0

    nc = bacc.Bacc("TRN2", target_bir_lowering=False, debug=False,
                   num_devices=N_CORES)

    lt = nc.dram_tensor("lt", [V, VLOC], mm_dt, kind="ExternalInput")
    xt = nc.dram_tensor("xt", [V, C_IN], mm_dt, kind="ExternalInput")
    xc = nc.dram_tensor("xc", [C_IN, VLOC], f32, kind="ExternalInput")
    wf = nc.dram_tensor("wf", [P, C_OUT], f32, kind="ExternalInput")
    w4 = nc.dram_tensor("w4", [C_IN, C_OUT], f32, kind="ExternalInput")
    bias_in = nc.dram_tensor("bias_in", [C_OUT, 1], f32, kind="ExternalInput")
    id128 = nc.dram_tensor("id128", [P, C_IN], f32, kind="ExternalInput")
    out = nc.dram_tensor("out", [C_OUT, VLOC], f32, kind="ExternalOutput")

    lt_r = lt.ap().rearrange("(g p u) c -> g p u c", p=P, u=NB)

    with tile.TileContext(nc) as tc:
        with (
            tc.tile_pool(name="ltp", bufs=LT_BUFS) as ltp,
            tc.tile_pool(name="persist", bufs=1) as persist,
            tc.tile_pool(name="stat", bufs=3 * NG_S) as statp,
            tc.tile_pool(name="work", bufs=2) as work,
            tc.tile_pool(name="acc", bufs=4, space="PSUM") as accp,
            tc.tile_pool(name="tpp", bufs=4, space="PSUM") as tpp,
            tc.tile_pool(name="dram", bufs=1, space="DRAM") as dram,
        ):
            # ---- persistent small tensors ----
            w_sb = persist.tile([P, C_OUT], f32)
            nc.scalar.dma_start(w_sb[:], wf.ap())
            w4_sb = persist.tile([C_IN, C_OUT], f32)
            nc.scalar.dma_start(w4_sb[:], w4.ap())
            bias_sb = persist.tile([C_OUT, 1], f32)
            nc.scalar.dma_start(bias_sb[:], bias_in.ap())
            id_sb = persist.tile([P, C_IN], f32)
            nc.scalar.dma_start(id_sb[:], id128.ap())

            # T_0..T_3 stacked at partition bases {0,32,64,96} of one tile
            # (32-aligned so every engine may address them); T_4 separate.
            t_blk = persist.tile([P, VLOC], f32)
            t4_sb = persist.tile([C_IN, VLOC], f32)
            nc.scalar.dma_start(t_blk[0:C_IN, :], xc.ap())

            def t_ap(k):
                if k == K_CHEB - 1:
                    return t4_sb[:]
                return t_blk[32 * k:32 * k + C_IN, :]

            # stationary tensors (x^T, then each gathered T_k^T) live in
            # rank-aligned 1536-row groups, row-interleaved so partition p
            # reads rows [g*1536 + 12p, +12) — one 384 B chunk.
            def load_stat(src_r, tag_k):
                tiles = []
                for g in range(NG_S):
                    s = statp.tile([P, NB_S * C_IN], mm_dt,
                                   name=f"st{tag_k}_{g}", tag="stat")
                    nc.scalar.dma_start(
                        s[:].rearrange("p (j c) -> p j c", j=NB_S), src_r[g])
                    tiles.append(s)
                return tiles

            xt_r = xt.ap().rearrange("(g p j) c -> g p j c", p=P, j=NB_S)
            sk_tiles = load_stat(xt_r, 0)

            # tiny warm-up AllGather: pays the ~70us first-collective cost
            # concurrently with step 1 instead of on the critical path
            wu_sb = work.tile([P, C_IN], mm_dt, name="wu_sb", tag="wu")
            nc.vector.memset(wu_sb[:], 0.0)
            for w in range(2):
                wu_in = dram.tile([P, C_IN], mm_dt, name=f"wu_in{w}")
                wu_out = dram.tile([P * N_CORES, C_IN], mm_dt,
                                   name=f"wu_out{w}")
                nc.scalar.dma_start(wu_in[:], wu_sb[:])
                nc.gpsimd.collective_compute(
                    "AllGather",
                    mybir.AluOpType.bypass,
                    replica_groups=[list(range(N_CORES))],
                    ins=[wu_in.opt()],
                    outs=[wu_out.opt()],
                )

            # resident lt: LAST RES_T vc-tiles, loaded once.  Sitting at the
            # end of each step, their matmuls need no DMA — covering the
            # transpose/all-gather chain while lt prefetches the next step.
            if res_g:
                rs_sb = persist.tile([P, RES_T * VLOC], mm_dt)
                for i, g in enumerate(range(NG - res_g, NG)):
                    nc.sync.dma_start(
                        rs_sb[:, i * NB * VLOC:(i + 1) * NB * VLOC]
                        .rearrange("p (u c) -> p u c", u=NB),
                        lt_r[g])

            for k in range(1, K_CHEB):
                acc = [accp.tile([C_IN, 512], f32, name=f"acc{k}_{ch}",
                                 tag="acc") for ch in range(N_CH)]
                # steps 1-3: resident groups last (their DMA-free matmuls
                # cover the all-gather chain while lt prefetches ahead);
                # step 4: resident second-to-last so the final streamed
                # group's DMA overlaps them and little trails the last byte
                if k < K_CHEB - 1 or not res_g:
                    g_order = list(range(NG))
                else:
                    ns = NG - res_g
                    g_order = list(range(ns - 1)) + \
                        list(range(ns, NG)) + [ns - 1]
                for gi, g in enumerate(g_order):
                    if g >= NG - res_g:
                        src = rs_sb
                        base = (g - (NG - res_g)) * NB * VLOC
                    else:
                        src = ltp.tile([P, NB * VLOC], mm_dt,
                                       name=f"lt{k}_{g}", tag="lt")
                        nc.sync.dma_start(
                            src[:].rearrange("p (u c) -> p u c", u=NB),
                            lt_r[g])
                        base = 0
                    for u in range(NB):
                        j = g * NB + u
                        st = sk_tiles[j // NB_S]
                        us = j % NB_S
                        for ch in range(N_CH):
                            nc.tensor.matmul(
                                acc[ch][:],
                                lhsT=st[:, us * C_IN:(us + 1) * C_IN],
                                rhs=src[:, base + u * VLOC + ch * 512:
                                        base + u * VLOC + (ch + 1) * 512],
                                start=(gi == 0 and u == 0),
                                stop=(gi == NG - 1 and u == NB - 1),
                            )
                    if k == 1 and g == NG // 2 - 1:
                        # third warm-up AG pinned mid-step-1, size-matched
                        # to the real all-gathers: re-syncs the ranks and
                        # warms the exact transfer shape
                        wu_sb2 = work.tile([P, NB_S * C_IN], mm_dt,
                                           name="wu_sb2", tag="scs")
                        nc.vector.tensor_copy(wu_sb2[:],
                                              src[:, :NB_S * C_IN])
                        wu_in2 = dram.tile([VLOC, C_IN], mm_dt,
                                           name="wu_in2")
                        wu_out2 = dram.tile([V, C_IN], mm_dt, name="wu_out2")
                        nc.scalar.dma_start(
                            wu_in2.rearrange("(p j) c -> p j c", p=P),
                            wu_sb2[:].rearrange("p (j c) -> p j c", j=NB_S))
                        nc.gpsimd.collective_compute(
                            "AllGather",
                            mybir.AluOpType.bypass,
                            replica_groups=[list(range(N_CORES))],
                            ins=[wu_in2.opt()],
                            outs=[wu_out2.opt()],
                        )
                # ---- T_k = 2*psum - T_{k-2}  (step 1: T_1 = psum) ----
                for ch in range(N_CH):
                    sl = slice(ch * 512, (ch + 1) * 512)
                    if k == 1:
                        nc.vector.tensor_copy(t_ap(k)[:, sl], acc[ch][:])
                    else:
                        nc.vector.scalar_tensor_tensor(
                            t_ap(k)[:, sl], acc[ch][:], 2.0,
                            t_ap(k - 2)[:, sl],
                            mybir.AluOpType.mult, mybir.AluOpType.subtract)

                if k < K_CHEB - 1:
                    # ---- transpose local chunk, cast, all-gather ----
                    sc_stage = work.tile([P, (VLOC // P) * C_IN], mm_dt,
                                         name=f"scs{k}", tag="scs")
                    tb = 32 * k
                    for j2 in range(VLOC // P):
                        tp_ps = tpp.tile([P, C_IN], f32, name=f"tp{k}_{j2}",
                                         tag="tp")
                        nc.tensor.transpose(
                            tp_ps[:],
                            t_blk[tb:tb + C_IN, j2 * P:(j2 + 1) * P],
                            id_sb[tb:tb + C_IN, :],
                            tile_position=(tb, 0) if tb == 96 else None)
                        nc.vector.tensor_copy(
                            sc_stage[:, j2 * C_IN:(j2 + 1) * C_IN], tp_ps[:])
                    cc_in = dram.tile([VLOC, C_IN], mm_dt, name=f"ccin{k}")
                    cc_out = dram.tile([V, C_IN], mm_dt, name=f"ccout{k}")
                    nc.scalar.dma_start(
                        cc_in.rearrange("(p j) c -> p j c", p=P),
                        sc_stage[:].rearrange("p (j c) -> p j c",
                                              j=VLOC // P))
                    nc.gpsimd.collective_compute(
                        "AllGather",
                        mybir.AluOpType.bypass,
                        replica_groups=[list(range(N_CORES))],
                        ins=[cc_in.opt()],
                        outs=[cc_out.opt()],
                    )
                    cc_r = cc_out.rearrange("(g p j) c -> g p j c",
                                            p=P, j=NB_S)
                    sk_tiles = load_stat(cc_r, k)

            # ---- out[o, v] = sum_k w_k^T @ T_k + bias ----
            # T_0..T_3 ride the four PE row-groups concurrently, each into
            # its own PSUM bank (concurrent row-group drains must not share
            # a bank); T_4 accumulates serially onto row-group 0's bank.
            for ch in range(N_CH):
                sl = slice(ch * 512, (ch + 1) * 512)
                ein = [accp.tile([C_OUT, 512], f32, name=f"ein{ch}_{k}",
                                 tag="acc") for k in range(K_CHEB - 1)]
                for k in range(K_CHEB - 1):
                    tb = 32 * k
                    nc.tensor.matmul(
                        ein[k][:],
                        lhsT=w_sb[tb:tb + C_IN, :],
                        rhs=t_blk[tb:tb + C_IN, sl],
                        start=True,
                        stop=(k != 0),
                        tile_position=(tb, 0) if tb == 96 else None,
                    )
                nc.tensor.matmul(ein[0][:], lhsT=w4_sb[:], rhs=t4_sb[:, sl],
                                 start=False, stop=True)
                # DVE reads at most one PSUM operand per op: chain the
                # reduction PSUM + SBUF -> SBUF.
                s0 = work.tile([C_OUT, 512], f32, name=f"s0_{ch}", tag="s0")
                nc.vector.tensor_scalar_add(s0[:], ein[0][:], bias_sb[:])
                s1 = work.tile([C_OUT, 512], f32, name=f"s1_{ch}", tag="s1")
                nc.vector.tensor_add(s1[:], ein[1][:], s0[:])
                s2 = work.tile([C_OUT, 512], f32, name=f"s2_{ch}", tag="s2")
                nc.vector.tensor_add(s2[:], ein[2][:], s1[:])
                res = work.tile([C_OUT, 512], f32, name=f"res{ch}", tag="res")
                nc.vector.tensor_add(res[:], ein[3][:], s2[:])
                nc.scalar.dma_start(out.ap()[:, sl], res[:])

    nc.compile()
    return nc


def _interleave_rows(a, nb):
    """Within each nb*128-row group, reorder rows so row g*G+nb*p+u holds
    original row g*G+u*128+p (one contiguous per-partition read)."""
    ng = a.shape[0] // (P * nb)
    return np.ascontiguousarray(
        a.reshape(ng, nb, P, a.shape[1]).transpose(0, 2, 1, 3)
        .reshape(a.shape))


def _prep_inputs(x, L, weights, bias, cfg: str):
    np_dt = ml_dtypes.bfloat16 if cfg == "bf16" else np.float32
    x = np.asarray(x, dtype=np.float32)
    L = np.asarray(L, dtype=np.float32)
    weights = np.asarray(weights, dtype=np.float32)
    bias = np.asarray(bias, dtype=np.float32)

    Lt = np.ascontiguousarray(L.T).astype(np_dt)          # (V, V)
    xt = _interleave_rows(
        np.ascontiguousarray(x.T).astype(np_dt), NB_S)    # (V, C_IN)

    wf = np.zeros((P, C_OUT), dtype=np.float32)
    for k in range(K_CHEB - 1):
        wf[32 * k:32 * k + C_IN, :] = weights[k]
    w4 = np.ascontiguousarray(weights[K_CHEB - 1])
    b_ = np.ascontiguousarray(bias.reshape(C_OUT, 1))
    id128 = np.zeros((P, C_IN), dtype=np.float32)
    for p in range(P):
        if p % 32 < C_IN:
            id128[p, p % 32] = 1.0

    in_maps = []
    for d in range(N_CORES):
        cols = slice(d * VLOC, (d + 1) * VLOC)
        in_maps.append({
            "lt": _interleave_rows(np.ascontiguousarray(Lt[:, cols]), NB),
            "xt": xt,
            "xc": np.ascontiguousarray(x[:, cols]),
            "wf": wf,
            "w4": w4,
            "bias_in": b_,
            "id128": id128,
        })
    return in_maps


def run(x, L, weights, bias, cfg: str = "bf16", trace: bool = False,
        trace_cores=None):
    if cfg not in _CACHE:
        _CACHE[cfg] = _build(cfg)
    nc = _CACHE[cfg]
    in_maps = _prep_inputs(x, L, weights, bias, cfg)
    kw = {}
    if trace_cores is not None:
        kw["trace_cores"] = trace_cores
    res = bass_utils.run_bass_kernel_spmd(
        nc, in_maps, core_ids=list(range(N_CORES)), trace=trace, **kw)
    out = np.concatenate([res.results[d]["out"] for d in range(N_CORES)],
                         axis=1)
    return out.astype(np.float32), res


def kernel(x, L, weights, bias):
    out, _ = run(x, L, weights, bias, cfg="bf16")
    return out



# revision 4
# speedup vs baseline: 1.0123x; 1.0123x over previous
import sys

if "/opt/trn_rl_repo" not in sys.path:
    sys.path.insert(0, "/opt/trn_rl_repo")

import numpy as np
import ml_dtypes

import concourse.bass as bass
import concourse.bacc as bacc
import concourse.tile as tile
import concourse.mybir as mybir
from concourse import bass_utils

# Problem shapes (nn_ChebConv): x (16, 12288), L (12288, 12288),
# weights (5, 16, 32), bias (32,). out (32, 12288).
#
# Sharding: core d owns V-columns [d*1536, (d+1)*1536).  Host feeds each
# core lt = L^T[:, cols_d] (contraction dim on partitions), row-interleaved
# within 256-row groups so each partition reads one contiguous chunk.
#
# Step k: psum(16,512)x3 accumulates T_{k-1} @ L^T over 96 vc-tiles.
# Half the lt slice (48 tiles) is resident in SBUF, loaded lazily during
# step 1 and reused DMA-free by steps 2-4; the other half re-streams each
# step.  Streamed and resident groups interleave so the PE is never
# DMA-paced; a resident-only tail at the end of each step is split by
# psum chunk so the transpose+AllGather of T_k[:, :1024] overlaps the
# chunk-2 matmuls, shrinking the step-boundary collective stall.
C_IN = 16
C_OUT = 32
K_CHEB = 5
V = 12288
N_CORES = 8
VLOC = V // N_CORES          # 1536 columns of the V axis per core
P = 128
NT_VC = V // P               # 96 contraction tiles per step
N_CH = VLOC // 512           # 3 psum chunks of 512
NB = 2                       # vc-tiles per lt DMA group (256 rows)
NG = NT_VC // NB             # 48 groups
NS = NG // 2                 # 24 streamed groups per step
NR = NG - NS                 # 24 resident groups
TAIL = 6                     # resident groups forming the chunk-split tail
LT_BUFS = 4
NB_S = VLOC // P             # 12 vc-tiles per stationary band
NG_S = V // (P * NB_S)       # 8 stationary bands
JA = 8                       # vc-tiles of a band covered by AG half 1

_CACHE: dict = {}


def _build(cfg: str):
    mm_dt = mybir.dt.bfloat16
    f32 = mybir.dt.float32

    nc = bacc.Bacc("TRN2", target_bir_lowering=False, debug=False,
                   num_devices=N_CORES)

    lt = nc.dram_tensor("lt", [V, VLOC], mm_dt, kind="ExternalInput")
    xt = nc.dram_tensor("xt", [V, C_IN], mm_dt, kind="ExternalInput")
    xc = nc.dram_tensor("xc", [C_IN, VLOC], f32, kind="ExternalInput")
    wf = nc.dram_tensor("wf", [P, C_OUT], f32, kind="ExternalInput")
    w4 = nc.dram_tensor("w4", [C_IN, C_OUT], f32, kind="ExternalInput")
    bias_in = nc.dram_tensor("bias_in", [C_OUT, 1], f32, kind="ExternalInput")
    id128 = nc.dram_tensor("id128", [P, C_IN], f32, kind="ExternalInput")
    out = nc.dram_tensor("out", [C_OUT, VLOC], f32, kind="ExternalOutput")

    lt_r = lt.ap().rearrange("(g p u) c -> g p u c", p=P, u=NB)

    with tile.TileContext(nc) as tc:
        with (
            tc.tile_pool(name="ltp", bufs=LT_BUFS) as ltp,
            tc.tile_pool(name="persist", bufs=1) as persist,
            tc.tile_pool(name="resp", bufs=1) as resp,
            tc.tile_pool(name="stat", bufs=3 * NG_S) as statp,
            tc.tile_pool(name="work", bufs=2) as work,
            tc.tile_pool(name="acc", bufs=4, space="PSUM") as accp,
            tc.tile_pool(name="tpp", bufs=4, space="PSUM") as tpp,
            tc.tile_pool(name="dram", bufs=1, space="DRAM") as dram,
        ):
            # ---- persistent small tensors ----
            w_sb = persist.tile([P, C_OUT], f32)
            nc.scalar.dma_start(w_sb[:], wf.ap())
            w4_sb = persist.tile([C_IN, C_OUT], f32)
            nc.scalar.dma_start(w4_sb[:], w4.ap())
            bias_sb = persist.tile([C_OUT, 1], f32)
            nc.scalar.dma_start(bias_sb[:], bias_in.ap())
            id_sb = persist.tile([P, C_IN], f32)
            nc.scalar.dma_start(id_sb[:], id128.ap())

            # T_0..T_3 stacked at partition bases {0,32,64,96} of one tile
            # (32-aligned so every engine may address them); T_4 separate.
            t_blk = persist.tile([P, VLOC], f32)
            t4_sb = persist.tile([C_IN, VLOC], f32)
            nc.scalar.dma_start(t_blk[0:C_IN, :], xc.ap())

            def t_ap(k):
                if k == K_CHEB - 1:
                    return t4_sb[:]
                return t_blk[32 * k:32 * k + C_IN, :]

            # stationary bands: band g holds T^T rows [g*1536, (g+1)*1536)
            # in plain j*128+p order; split A (j<8) / B (j>=8) so step k+1
            # matmuls on A-tiles need only the first AllGather half.
            xt_r = xt.ap().rearrange("(g j p) c -> g p j c", p=P, j=NB_S)
            sA, sB = [], []
            for g in range(NG_S):
                a = statp.tile([P, JA * C_IN], mm_dt, name=f"sA0_{g}",
                               tag="stA")
                nc.scalar.dma_start(
                    a[:].rearrange("p (j c) -> p j c", j=JA),
                    xt_r[g, :, 0:JA])
                b = statp.tile([P, (NB_S - JA) * C_IN], mm_dt,
                               name=f"sB0_{g}", tag="stB")
                nc.scalar.dma_start(
                    b[:].rearrange("p (j c) -> p j c", j=NB_S - JA),
                    xt_r[g, :, JA:NB_S])
                sA.append(a)
                sB.append(b)

            def st_ap(j):
                g, jj = j // NB_S, j % NB_S
                if jj < JA:
                    return sA[g][:, jj * C_IN:(jj + 1) * C_IN]
                return sB[g][:, (jj - JA) * C_IN:(jj - JA + 1) * C_IN]

            # tiny warm-up AllGathers: pay the first-collective cost
            # during step 1's DMA-bound phase, off the critical path
            wu_sb = work.tile([P, C_IN], mm_dt, name="wu_sb", tag="wu")
            nc.vector.memset(wu_sb[:], 0.0)
            for w in range(2):
                wu_in = dram.tile([P, C_IN], mm_dt, name=f"wu_in{w}")
                wu_out = dram.tile([P * N_CORES, C_IN], mm_dt,
                                   name=f"wu_out{w}")
                nc.scalar.dma_start(wu_in[:], wu_sb[:])
                nc.gpsimd.collective_compute(
                    "AllGather",
                    mybir.AluOpType.bypass,
                    replica_groups=[list(range(N_CORES))],
                    ins=[wu_in.opt()],
                    outs=[wu_out.opt()],
                )

            res_tiles = [None] * NR

            def lt_src(g, k):
                """SBUF tile + DMA for lt group g (0..NG-1) in step k."""
                if g >= NS:
                    ri = g - NS
                    if res_tiles[ri] is None:
                        t = resp.tile([P, NB * VLOC], mm_dt, name=f"res{ri}")
                        nc.sync.dma_start(
                            t[:].rearrange("p (u c) -> p u c", u=NB),
                            lt_r[g])
                        res_tiles[ri] = t
                    return res_tiles[ri]
                t = ltp.tile([P, NB * VLOC], mm_dt, name=f"lt{k}_{g}",
                             tag="lt")
                nc.sync.dma_start(
                    t[:].rearrange("p (u c) -> p u c", u=NB), lt_r[g])
                return t

            def mm_group(src, g, ch_list, acc, started, g_last):
                for u in range(NB):
                    j = g * NB + u
                    for ch in ch_list:
                        nc.tensor.matmul(
                            acc[ch][:],
                            lhsT=st_ap(j),
                            rhs=src[:, u * VLOC + ch * 512:
                                    u * VLOC + (ch + 1) * 512],
                            start=(ch not in started),
                            stop=(g == g_last and u == NB - 1),
                        )
                        started.add(ch)

            def drain(k, ch_list, acc):
                """T_k chunks -> transpose -> stage for AllGather half."""
                for ch in ch_list:
                    sl = slice(ch * 512, (ch + 1) * 512)
                    if k == 1:
                        nc.vector.tensor_copy(t_ap(k)[:, sl], acc[ch][:])
                    else:
                        nc.vector.scalar_tensor_tensor(
                            t_ap(k)[:, sl], acc[ch][:], 2.0,
                            t_ap(k - 2)[:, sl],
                            mybir.AluOpType.mult, mybir.AluOpType.subtract)

            for k in range(1, K_CHEB):
                acc = [accp.tile([C_IN, 512], f32, name=f"acc{k}_{ch}",
                                 tag="acc") for ch in range(N_CH)]
                started = set()
                # main phase: interleave streamed/resident so the PE is
                # never DMA-paced; step 1 is DMA-bound anyway so it just
                # streams in order.
                if k == 1:
                    main = list(range(NS)) + \
                        [NS + r for r in range(NR - TAIL)]
                else:
                    main = []
                    for i in range(NS):
                        main.append(i)
                        if i < NR - TAIL:
                            main.append(NS + i)
                tail = [NS + r for r in range(NR - TAIL, NR)]

                for gi, g in enumerate(main):
                    src = lt_src(g, k)
                    mm_group(src, g, range(N_CH), acc, started, tail[-1])
                    if k == 1 and g == NS // 2:
                        # third warm-up AG pinned mid-step-1, size-matched
                        # to the real first AG half
                        wu_sb2 = work.tile([P, JA * C_IN], mm_dt,
                                           name="wu_sb2", tag="scs")
                        nc.vector.tensor_copy(wu_sb2[:],
                                              src[:, :JA * C_IN])
                        wu_in2 = dram.tile([JA * P, C_IN], mm_dt,
                                           name="wu_in2")
                        wu_out2 = dram.tile([JA * P * N_CORES, C_IN],
                                            mm_dt, name="wu_out2")
                        nc.scalar.dma_start(
                            wu_in2.rearrange("(j p) c -> p j c", p=P),
                            wu_sb2[:].rearrange("p (j c) -> p j c", j=JA))
                        nc.gpsimd.collective_compute(
                            "AllGather",
                            mybir.AluOpType.bypass,
                            replica_groups=[list(range(N_CORES))],
                            ins=[wu_in2.opt()],
                            outs=[wu_out2.opt()],
                        )

                # tail phase A: chunks {0,1} of the last TAIL resident
                # groups, then drain+gather T_k[:, :1024] while phase B
                # (chunk 2) still runs on the PE.
                for g in tail:
                    mm_group(lt_src(g, k), g, range(N_CH - 1), acc,
                             started, tail[-1])
                drain(k, range(N_CH - 1), acc)

                last = k == K_CHEB - 1
                if not last:
                    sc_stage = work.tile([P, NB_S * C_IN], mm_dt,
                                         name=f"scs{k}", tag="scs")
                    tb = 32 * k

                    def xpose(j2):
                        tp_ps = tpp.tile([P, C_IN], f32, name=f"tp{k}_{j2}",
                                         tag="tp")
                        nc.tensor.transpose(
                            tp_ps[:],
                            t_blk[tb:tb + C_IN, j2 * P:(j2 + 1) * P],
                            id_sb[tb:tb + C_IN, :],
                            tile_position=(tb, 0) if tb == 96 else None)
                        nc.vector.tensor_copy(
                            sc_stage[:, j2 * C_IN:(j2 + 1) * C_IN],
                            tp_ps[:])

                    for j2 in range(JA):
                        xpose(j2)
                    cc_in1 = dram.tile([JA * P, C_IN], mm_dt,
                                       name=f"ccin{k}a")
                    cc_out1 = dram.tile([JA * P * N_CORES, C_IN], mm_dt,
                                        name=f"ccout{k}a")
                    nc.scalar.dma_start(
                        cc_in1.rearrange("(j p) c -> p j c", p=P),
                        sc_stage[:, :JA * C_IN]
                        .rearrange("p (j c) -> p j c", j=JA))
                    nc.gpsimd.collective_compute(
                        "AllGather",
                        mybir.AluOpType.bypass,
                        replica_groups=[list(range(N_CORES))],
                        ins=[cc_in1.opt()],
                        outs=[cc_out1.opt()],
                    )

                # tail phase B: chunk 2 matmuls, drain, second AG half
                for g in tail:
                    mm_group(lt_src(g, k), g, [N_CH - 1], acc, started,
                             tail[-1])
                drain(k, [N_CH - 1], acc)

                if not last:
                    for j2 in range(JA, NB_S):
                        xpose(j2)
                    JB = NB_S - JA
                    cc_in2 = dram.tile([JB * P, C_IN], mm_dt,
                                       name=f"ccin{k}b")
                    cc_out2 = dram.tile([JB * P * N_CORES, C_IN], mm_dt,
                                        name=f"ccout{k}b")
                    nc.scalar.dma_start(
                        cc_in2.rearrange("(j p) c -> p j c", p=P),
                        sc_stage[:, JA * C_IN:]
                        .rearrange("p (j c) -> p j c", j=JB))
                    nc.gpsimd.collective_compute(
                        "AllGather",
                        mybir.AluOpType.bypass,
                        replica_groups=[list(range(N_CORES))],
                        ins=[cc_in2.opt()],
                        outs=[cc_out2.opt()],
                    )
                    # next step's stationary bands
                    cc1_r = cc_out1.rearrange("(g j p) c -> g p j c",
                                              p=P, j=JA)
                    cc2_r = cc_out2.rearrange("(g j p) c -> g p j c",
                                              p=P, j=JB)
                    sA, sB = [], []
                    for g in range(NG_S):
                        a = statp.tile([P, JA * C_IN], mm_dt,
                                       name=f"sA{k}_{g}", tag="stA")
                        nc.scalar.dma_start(
                            a[:].rearrange("p (j c) -> p j c", j=JA),
                            cc1_r[g])
                        b = statp.tile([P, JB * C_IN], mm_dt,
                                       name=f"sB{k}_{g}", tag="stB")
                        nc.scalar.dma_start(
                            b[:].rearrange("p (j c) -> p j c", j=JB),
                            cc2_r[g])
                        sA.append(a)
                        sB.append(b)

            # ---- out[o, v] = sum_k w_k^T @ T_k + bias ----
            # T_0..T_3 ride the four PE row-groups concurrently, each into
            # its own PSUM bank (concurrent row-group drains must not share
            # a bank); T_4 accumulates serially onto row-group 0's bank.
            for ch in range(N_CH):
                sl = slice(ch * 512, (ch + 1) * 512)
                ein = [accp.tile([C_OUT, 512], f32, name=f"ein{ch}_{k}",
                                 tag="acc") for k in range(K_CHEB - 1)]
                for k in range(K_CHEB - 1):
                    tb = 32 * k
                    nc.tensor.matmul(
                        ein[k][:],
                        lhsT=w_sb[tb:tb + C_IN, :],
                        rhs=t_blk[tb:tb + C_IN, sl],
                        start=True,
                        stop=(k != 0),
                        tile_position=(tb, 0) if tb == 96 else None,
                    )
                nc.tensor.matmul(ein[0][:], lhsT=w4_sb[:], rhs=t4_sb[:, sl],
                                 start=False, stop=True)
                # DVE reads at most one PSUM operand per op: chain the
                # reduction PSUM + SBUF -> SBUF.
                s0 = work.tile([C_OUT, 512], f32, name=f"s0_{ch}", tag="s0")
                nc.vector.tensor_scalar_add(s0[:], ein[0][:], bias_sb[:])
                s1 = work.tile([C_OUT, 512], f32, name=f"s1_{ch}", tag="s1")
                nc.vector.tensor_add(s1[:], ein[1][:], s0[:])
                s2 = work.tile([C_OUT, 512], f32, name=f"s2_{ch}", tag="s2")
                nc.vector.tensor_add(s2[:], ein[2][:], s1[:])
                res = work.tile([C_OUT, 512], f32, name=f"res{ch}", tag="res")
                nc.vector.tensor_add(res[:], ein[3][:], s2[:])
                nc.scalar.dma_start(out.ap()[:, sl], res[:])

    nc.compile()
    return nc


def _interleave_rows(a, nb):
    """Within each nb*128-row group, reorder rows so row g*G+nb*p+u holds
    original row g*G+u*128+p (one contiguous per-partition read)."""
    ng = a.shape[0] // (P * nb)
    return np.ascontiguousarray(
        a.reshape(ng, nb, P, a.shape[1]).transpose(0, 2, 1, 3)
        .reshape(a.shape))


def _prep_inputs(x, L, weights, bias, cfg: str):
    np_dt = ml_dtypes.bfloat16
    x = np.asarray(x, dtype=np.float32)
    L = np.asarray(L, dtype=np.float32)
    weights = np.asarray(weights, dtype=np.float32)
    bias = np.asarray(bias, dtype=np.float32)

    Lt = np.ascontiguousarray(L.T).astype(np_dt)          # (V, V)
    xt = np.ascontiguousarray(x.T).astype(np_dt)          # (V, C_IN)

    wf = np.zeros((P, C_OUT), dtype=np.float32)
    for k in range(K_CHEB - 1):
        wf[32 * k:32 * k + C_IN, :] = weights[k]
    w4 = np.ascontiguousarray(weights[K_CHEB - 1])
    b_ = np.ascontiguousarray(bias.reshape(C_OUT, 1))
    id128 = np.zeros((P, C_IN), dtype=np.float32)
    for p in range(P):
        if p % 32 < C_IN:
            id128[p, p % 32] = 1.0

    in_maps = []
    for d in range(N_CORES):
        cols = slice(d * VLOC, (d + 1) * VLOC)
        in_maps.append({
            "lt": _interleave_rows(np.ascontiguousarray(Lt[:, cols]), NB),
            "xt": xt,
            "xc": np.ascontiguousarray(x[:, cols]),
            "wf": wf,
            "w4": w4,
            "bias_in": b_,
            "id128": id128,
        })
    return in_maps


def run(x, L, weights, bias, cfg: str = "bf16", trace: bool = False,
        trace_cores=None):
    if cfg not in _CACHE:
        _CACHE[cfg] = _build(cfg)
    nc = _CACHE[cfg]
    in_maps = _prep_inputs(x, L, weights, bias, cfg)
    kw = {}
    if trace_cores is not None:
        kw["trace_cores"] = trace_cores
    res = bass_utils.run_bass_kernel_spmd(
        nc, in_maps, core_ids=list(range(N_CORES)), trace=trace, **kw)
    out = np.concatenate([res.results[d]["out"] for d in range(N_CORES)],
                         axis=1)
    return out.astype(np.float32), res


def kernel(x, L, weights, bias):
    out, _ = run(x, L, weights, bias, cfg="bf16")
    return out


# revision 8
# speedup vs baseline: 1.0295x; 1.0170x over previous
import sys

if "/opt/trn_rl_repo" not in sys.path:
    sys.path.insert(0, "/opt/trn_rl_repo")

import numpy as np
import ml_dtypes

import concourse.bass as bass
import concourse.bacc as bacc
import concourse.tile as tile
import concourse.mybir as mybir
from concourse import bass_utils

# Problem shapes (nn_ChebConv): x (16, 12288), L (12288, 12288),
# weights (5, 16, 32), bias (32,). out (32, 12288).
#
# Sharding: core d owns V-columns [d*1536, (d+1)*1536).  Host feeds each
# core lt = L^T[:, cols_d] (contraction dim on partitions), row-interleaved
# within 256-row groups so each partition reads one contiguous chunk.
#
# Step k: psum(16,512)x3 accumulates T_{k-1} @ L^T over 96 vc-tiles.
# Half the lt slice (48 tiles) is resident in SBUF, loaded lazily during
# step 1 and reused DMA-free by steps 2-4; the other half re-streams each
# step.  Streamed and resident groups interleave so the PE is never
# DMA-paced; a resident-only tail at the end of each step is split by
# psum chunk so the transpose+AllGather of T_k[:, :1024] overlaps the
# chunk-2 matmuls, shrinking the step-boundary collective stall.
C_IN = 16
C_OUT = 32
K_CHEB = 5
V = 12288
N_CORES = 8
VLOC = V // N_CORES          # 1536 columns of the V axis per core
P = 128
NT_VC = V // P               # 96 contraction tiles per step
N_CH = VLOC // 512           # 3 psum chunks of 512
NB = 2                       # vc-tiles per lt DMA group (256 rows)
NG = NT_VC // NB             # 48 groups
NS = NG // 2                 # 24 streamed groups per step
NR = NG - NS                 # 24 resident groups
TAIL = 6                     # resident groups forming the chunk-split tail
LT_BUFS = 4
NB_S = VLOC // P             # 12 vc-tiles per stationary band
NG_S = V // (P * NB_S)       # 8 stationary bands
JA = 8                       # vc-tiles of a band covered by AG half 1

_CACHE: dict = {}


def _build(cfg: str):
    mm_dt = mybir.dt.bfloat16
    f32 = mybir.dt.float32

    nc = bacc.Bacc("TRN2", target_bir_lowering=False, debug=False,
                   num_devices=N_CORES)

    lt = nc.dram_tensor("lt", [V, VLOC], mm_dt, kind="ExternalInput")
    xt = nc.dram_tensor("xt", [V, C_IN], mm_dt, kind="ExternalInput")
    xc = nc.dram_tensor("xc", [C_IN, VLOC], f32, kind="ExternalInput")
    wf = nc.dram_tensor("wf", [P, C_OUT], f32, kind="ExternalInput")
    w4 = nc.dram_tensor("w4", [C_IN, C_OUT], f32, kind="ExternalInput")
    bias_in = nc.dram_tensor("bias_in", [C_OUT, 1], f32, kind="ExternalInput")
    id128 = nc.dram_tensor("id128", [P, C_IN], f32, kind="ExternalInput")
    out = nc.dram_tensor("out", [C_OUT, VLOC], f32, kind="ExternalOutput")

    lt_r = lt.ap().rearrange("(g p u) c -> g p u c", p=P, u=NB)

    with tile.TileContext(nc) as tc:
        with (
            tc.tile_pool(name="ltp", bufs=LT_BUFS) as ltp,
            tc.tile_pool(name="persist", bufs=1) as persist,
            tc.tile_pool(name="resp", bufs=1) as resp,
            tc.tile_pool(name="stat", bufs=3 * NG_S) as statp,
            tc.tile_pool(name="work", bufs=2) as work,
            tc.tile_pool(name="acc", bufs=4, space="PSUM") as accp,
            tc.tile_pool(name="tpp", bufs=4, space="PSUM") as tpp,
            tc.tile_pool(name="dram", bufs=1, space="DRAM") as dram,
        ):
            # ---- persistent small tensors ----
            w_sb = persist.tile([P, C_OUT], f32)
            nc.scalar.dma_start(w_sb[:], wf.ap())
            w4_sb = persist.tile([C_IN, C_OUT], f32)
            nc.scalar.dma_start(w4_sb[:], w4.ap())
            bias_sb = persist.tile([C_OUT, 1], f32)
            nc.scalar.dma_start(bias_sb[:], bias_in.ap())
            id_sb = persist.tile([P, C_IN], f32)
            nc.scalar.dma_start(id_sb[:], id128.ap())

            # T_0..T_3 stacked at partition bases {0,32,64,96} of one tile
            # (32-aligned so every engine may address them); T_4 separate.
            t_blk = persist.tile([P, VLOC], f32)
            t4_sb = persist.tile([C_IN, VLOC], f32)
            nc.scalar.dma_start(t_blk[0:C_IN, :], xc.ap())

            def t_ap(k):
                if k == K_CHEB - 1:
                    return t4_sb[:]
                return t_blk[32 * k:32 * k + C_IN, :]

            # stationary bands: band g holds T^T rows [g*1536, (g+1)*1536)
            # in plain j*128+p order; split A (j<8) / B (j>=8) so step k+1
            # matmuls on A-tiles need only the first AllGather half.
            xt_r = xt.ap().rearrange("(g j p) c -> g p j c", p=P, j=NB_S)
            sk = []
            for g in range(NG_S):
                a = statp.tile([P, NB_S * C_IN], mm_dt, name=f"st0_{g}",
                               tag="stat")
                nc.scalar.dma_start(
                    a[:].rearrange("p (j c) -> p j c", j=NB_S), xt_r[g])
                sk.append(a)

            def st_ap(j):
                g, jj = j // NB_S, j % NB_S
                return sk[g][:, jj * C_IN:(jj + 1) * C_IN]

            # tiny warm-up AllGathers: pay the first-collective cost
            # during step 1's DMA-bound phase, off the critical path
            wu_sb = work.tile([P, C_IN], mm_dt, name="wu_sb", tag="wu")
            nc.vector.memset(wu_sb[:], 0.0)
            for w in range(2):
                wu_in = dram.tile([P, C_IN], mm_dt, name=f"wu_in{w}")
                wu_out = dram.tile([P * N_CORES, C_IN], mm_dt,
                                   name=f"wu_out{w}")
                nc.scalar.dma_start(wu_in[:], wu_sb[:])
                nc.gpsimd.collective_compute(
                    "AllGather",
                    mybir.AluOpType.bypass,
                    replica_groups=[list(range(N_CORES))],
                    ins=[wu_in.opt()],
                    outs=[wu_out.opt()],
                )

            res_tiles = [None] * NR

            def lt_src(g, k):
                """SBUF tile + DMA for lt group g (0..NG-1) in step k."""
                if g >= NS:
                    ri = g - NS
                    if res_tiles[ri] is None:
                        t = resp.tile([P, NB * VLOC], mm_dt, name=f"res{ri}")
                        nc.sync.dma_start(
                            t[:].rearrange("p (u c) -> p u c", u=NB),
                            lt_r[g])
                        res_tiles[ri] = t
                    return res_tiles[ri]
                t = ltp.tile([P, NB * VLOC], mm_dt, name=f"lt{k}_{g}",
                             tag="lt")
                nc.sync.dma_start(
                    t[:].rearrange("p (u c) -> p u c", u=NB), lt_r[g])
                return t

            def mm_group(src, g, ch_list, acc, started, g_last):
                for u in range(NB):
                    j = g * NB + u
                    for ch in ch_list:
                        nc.tensor.matmul(
                            acc[ch][:],
                            lhsT=st_ap(j),
                            rhs=src[:, u * VLOC + ch * 512:
                                    u * VLOC + (ch + 1) * 512],
                            start=(ch not in started),
                            stop=(g == g_last and u == NB - 1),
                        )
                        started.add(ch)

            def drain(k, ch_list, acc):
                """T_k chunks -> transpose -> stage for AllGather half."""
                for ch in ch_list:
                    sl = slice(ch * 512, (ch + 1) * 512)
                    if k == 1:
                        nc.vector.tensor_copy(t_ap(k)[:, sl], acc[ch][:])
                    else:
                        nc.vector.scalar_tensor_tensor(
                            t_ap(k)[:, sl], acc[ch][:], 2.0,
                            t_ap(k - 2)[:, sl],
                            mybir.AluOpType.mult, mybir.AluOpType.subtract)

            # out[o, v] = sum_k w_k^T @ T_k + bias, one 512-col chunk.
            # T_0..T_3 ride the four PE row-groups concurrently, each into
            # its own PSUM bank (concurrent row-group drains must not
            # share a bank); T_4 accumulates serially onto row-group 0's.
            def einsum(ch):
                sl = slice(ch * 512, (ch + 1) * 512)
                ein = [accp.tile([C_OUT, 512], f32, name=f"ein{ch}_{kk}",
                                 tag="acc") for kk in range(K_CHEB - 1)]
                for kk in range(K_CHEB - 1):
                    tb = 32 * kk
                    nc.tensor.matmul(
                        ein[kk][:],
                        lhsT=w_sb[tb:tb + C_IN, :],
                        rhs=t_blk[tb:tb + C_IN, sl],
                        start=True,
                        stop=(kk != 0),
                        tile_position=(tb, 0) if tb == 96 else None,
                    )
                nc.tensor.matmul(ein[0][:], lhsT=w4_sb[:], rhs=t4_sb[:, sl],
                                 start=False, stop=True)
                # DVE reads at most one PSUM operand per op: chain the
                # reduction PSUM + SBUF -> SBUF.
                s0 = work.tile([C_OUT, 512], f32, name=f"s0_{ch}", tag="s0")
                nc.vector.tensor_scalar_add(s0[:], ein[0][:], bias_sb[:])
                s1 = work.tile([C_OUT, 512], f32, name=f"s1_{ch}", tag="s1")
                nc.vector.tensor_add(s1[:], ein[1][:], s0[:])
                s2 = work.tile([C_OUT, 512], f32, name=f"s2_{ch}", tag="s2")
                nc.vector.tensor_add(s2[:], ein[2][:], s1[:])
                res = work.tile([C_OUT, 512], f32, name=f"res{ch}",
                                tag="res")
                nc.vector.tensor_add(res[:], ein[3][:], s2[:])
                nc.scalar.dma_start(out.ap()[:, sl], res[:])

            for k in range(1, K_CHEB):
                acc = [accp.tile([C_IN, 512], f32, name=f"acc{k}_{ch}",
                                 tag="acc") for ch in range(N_CH)]
                started = set()
                # main phase: Bresenham-interleave streamed/resident so
                # streamed-group consumption stays below DMA delivery rate
                # across the whole step; step 1 is DMA-bound anyway so it
                # just streams in order.
                if k == 1:
                    main = list(range(NS)) + \
                        [NS + r for r in range(NR - TAIL)]
                else:
                    main = []
                    si, ri = 0, 0
                    nmain = NS + NR - TAIL
                    for i in range(nmain):
                        if si * (nmain) <= i * NS and si < NS:
                            main.append(si)
                            si += 1
                        else:
                            main.append(NS + ri)
                            ri += 1
                tail = [NS + r for r in range(NR - TAIL, NR)]

                for gi, g in enumerate(main):
                    src = lt_src(g, k)
                    mm_group(src, g, range(N_CH), acc, started, tail[-1])
                    if k == 1 and g == NS // 2:
                        # third warm-up AG pinned mid-step-1, size-matched
                        # to the real first AG half
                        wu_sb2 = work.tile([P, NB_S * C_IN], mm_dt,
                                           name="wu_sb2", tag="scs")
                        nc.vector.tensor_copy(wu_sb2[:],
                                              src[:, :NB_S * C_IN])
                        wu_in2 = dram.tile([NB_S * P, C_IN], mm_dt,
                                           name="wu_in2")
                        wu_out2 = dram.tile([NB_S * P * N_CORES, C_IN],
                                            mm_dt, name="wu_out2")
                        nc.scalar.dma_start(
                            wu_in2.rearrange("(j p) c -> p j c", p=P),
                            wu_sb2[:].rearrange("p (j c) -> p j c", j=NB_S))
                        nc.gpsimd.collective_compute(
                            "AllGather",
                            mybir.AluOpType.bypass,
                            replica_groups=[list(range(N_CORES))],
                            ins=[wu_in2.opt()],
                            outs=[wu_out2.opt()],
                        )

                # tail: last TAIL resident groups run chunks {0,1}
                # first, so T_k[:, :1024] drains while chunk-2 matmuls
                # still occupy the PE (overlaps einsum in step 4).
                for g in tail:
                    mm_group(lt_src(g, k), g, range(N_CH - 1), acc,
                             started, tail[-1])
                drain(k, range(N_CH - 1), acc)

                last = k == K_CHEB - 1
                if last:
                    for ch in range(N_CH - 1):
                        einsum(ch)
                for g in tail:
                    mm_group(lt_src(g, k), g, [N_CH - 1], acc, started,
                             tail[-1])
                drain(k, [N_CH - 1], acc)
                if last:
                    einsum(N_CH - 1)
                    continue

                # transpose T_k^T and all-gather it for the next step's
                # stationary bands (one collective; latency is fixed-cost
                # and skew dominated, so splitting it does not pay).
                sc_stage = work.tile([P, NB_S * C_IN], mm_dt,
                                     name=f"scs{k}", tag="scs")
                tb = 32 * k
                for j2 in range(NB_S):
                    tp_ps = tpp.tile([P, C_IN], f32, name=f"tp{k}_{j2}",
                                     tag="tp")
                    nc.tensor.transpose(
                        tp_ps[:],
                        t_blk[tb:tb + C_IN, j2 * P:(j2 + 1) * P],
                        id_sb[tb:tb + C_IN, :],
                        tile_position=(tb, 0) if tb == 96 else None)
                    nc.vector.tensor_copy(
                        sc_stage[:, j2 * C_IN:(j2 + 1) * C_IN], tp_ps[:])
                cc_in = dram.tile([VLOC, C_IN], mm_dt, name=f"ccin{k}")
                cc_out = dram.tile([V, C_IN], mm_dt, name=f"ccout{k}")
                nc.scalar.dma_start(
                    cc_in.rearrange("(j p) c -> p j c", p=P),
                    sc_stage[:].rearrange("p (j c) -> p j c", j=NB_S))
                nc.gpsimd.collective_compute(
                    "AllGather",
                    mybir.AluOpType.bypass,
                    replica_groups=[list(range(N_CORES))],
                    ins=[cc_in.opt()],
                    outs=[cc_out.opt()],
                )
                cc_r = cc_out.rearrange("(g j p) c -> g p j c",
                                        p=P, j=NB_S)
                sk = []
                for g in range(NG_S):
                    a = statp.tile([P, NB_S * C_IN], mm_dt,
                                   name=f"st{k}_{g}", tag="stat")
                    nc.scalar.dma_start(
                        a[:].rearrange("p (j c) -> p j c", j=NB_S),
                        cc_r[g])
                    sk.append(a)

    nc.compile()
    return nc


def _interleave_rows(a, nb):
    """Within each nb*128-row group, reorder rows so row g*G+nb*p+u holds
    original row g*G+u*128+p (one contiguous per-partition read)."""
    ng = a.shape[0] // (P * nb)
    return np.ascontiguousarray(
        a.reshape(ng, nb, P, a.shape[1]).transpose(0, 2, 1, 3)
        .reshape(a.shape))


def _prep_inputs(x, L, weights, bias, cfg: str):
    np_dt = ml_dtypes.bfloat16
    x = np.asarray(x, dtype=np.float32)
    L = np.asarray(L, dtype=np.float32)
    weights = np.asarray(weights, dtype=np.float32)
    bias = np.asarray(bias, dtype=np.float32)

    Lt = np.ascontiguousarray(L.T).astype(np_dt)          # (V, V)
    xt = np.ascontiguousarray(x.T).astype(np_dt)          # (V, C_IN)

    wf = np.zeros((P, C_OUT), dtype=np.float32)
    for k in range(K_CHEB - 1):
        wf[32 * k:32 * k + C_IN, :] = weights[k]
    w4 = np.ascontiguousarray(weights[K_CHEB - 1])
    b_ = np.ascontiguousarray(bias.reshape(C_OUT, 1))
    id128 = np.zeros((P, C_IN), dtype=np.float32)
    for p in range(P):
        if p % 32 < C_IN:
            id128[p, p % 32] = 1.0

    in_maps = []
    for d in range(N_CORES):
        cols = slice(d * VLOC, (d + 1) * VLOC)
        in_maps.append({
            "lt": _interleave_rows(np.ascontiguousarray(Lt[:, cols]), NB),
            "xt": xt,
            "xc": np.ascontiguousarray(x[:, cols]),
            "wf": wf,
            "w4": w4,
            "bias_in": b_,
            "id128": id128,
        })
    return in_maps


def run(x, L, weights, bias, cfg: str = "bf16", trace: bool = False,
        trace_cores=None):
    if cfg not in _CACHE:
        _CACHE[cfg] = _build(cfg)
    nc = _CACHE[cfg]
    in_maps = _prep_inputs(x, L, weights, bias, cfg)
    kw = {}
    if trace_cores is not None:
        kw["trace_cores"] = trace_cores
    res = bass_utils.run_bass_kernel_spmd(
        nc, in_maps, core_ids=list(range(N_CORES)), trace=trace, **kw)
    out = np.concatenate([res.results[d]["out"] for d in range(N_CORES)],
                         axis=1)
    return out.astype(np.float32), res


def kernel(x, L, weights, bias):
    out, _ = run(x, L, weights, bias, cfg="bf16")
    return out


# revision 10
# speedup vs baseline: 1.1660x; 1.1326x over previous
import sys

if "/opt/trn_rl_repo" not in sys.path:
    sys.path.insert(0, "/opt/trn_rl_repo")

import numpy as np
import ml_dtypes

import concourse.bass as bass
import concourse.bacc as bacc
import concourse.tile as tile
import concourse.mybir as mybir
from concourse import bass_utils

# Problem shapes (nn_ChebConv): x (16, 12288), L (12288, 12288),
# weights (5, 16, 32), bias (32,). out (32, 12288).
#
# Sharding: core d owns V-columns [d*1536, (d+1)*1536).  Host feeds each
# core lt = L^T[:, cols_d] (contraction dim on partitions), row-interleaved
# within 256-row groups so each partition reads one contiguous chunk.
#
# Step k: psum(16,512)x3 accumulates T_{k-1} @ L^T over 96 vc-tiles.
# Half the lt slice (48 tiles) is resident in SBUF, loaded lazily during
# step 1 and reused DMA-free by steps 2-4; the other half re-streams each
# step.  Streamed and resident groups interleave so the PE is never
# DMA-paced; a resident-only tail at the end of each step is split by
# psum chunk so the transpose+AllGather of T_k[:, :1024] overlaps the
# chunk-2 matmuls, shrinking the step-boundary collective stall.
C_IN = 16
C_OUT = 32
K_CHEB = 5
V = 12288
N_CORES = 8
VLOC = V // N_CORES          # 1536 columns of the V axis per core
P = 128
NT_VC = V // P               # 96 contraction tiles per step
N_CH = VLOC // 512           # 3 psum chunks of 512
NB = 2                       # vc-tiles per lt DMA group (256 rows)
NG = NT_VC // NB             # 48 groups
NS = NG // 2                 # 24 streamed groups per step
NR = NG - NS                 # 24 resident groups
TAIL = 6                     # resident groups forming the chunk-split tail
LT_BUFS = 4
NB_S = VLOC // P             # 12 vc-tiles per stationary band
NG_S = V // (P * NB_S)       # 8 stationary bands
JA = 8                       # vc-tiles of a band covered by AG half 1

_CACHE: dict = {}


def _build(cfg: str):
    mm_dt = mybir.dt.bfloat16
    f32 = mybir.dt.float32

    nc = bacc.Bacc("TRN2", target_bir_lowering=False, debug=False,
                   num_devices=N_CORES)

    lt = nc.dram_tensor("lt", [V, VLOC], mm_dt, kind="ExternalInput")
    xt = nc.dram_tensor("xt", [V, C_IN], mm_dt, kind="ExternalInput")
    xc = nc.dram_tensor("xc", [C_IN, VLOC], f32, kind="ExternalInput")
    wf = nc.dram_tensor("wf", [P, C_OUT], f32, kind="ExternalInput")
    w4 = nc.dram_tensor("w4", [C_IN, C_OUT], f32, kind="ExternalInput")
    bias_in = nc.dram_tensor("bias_in", [C_OUT, 1], f32, kind="ExternalInput")
    id128 = nc.dram_tensor("id128", [P, C_IN], f32, kind="ExternalInput")
    out = nc.dram_tensor("out", [C_OUT, VLOC], f32, kind="ExternalOutput")

    lt_r = lt.ap().rearrange("(g p u) c -> g p u c", p=P, u=NB)

    with tile.TileContext(nc) as tc:
        with (
            tc.tile_pool(name="ltp", bufs=LT_BUFS) as ltp,
            tc.tile_pool(name="persist", bufs=1) as persist,
            tc.tile_pool(name="resp", bufs=1) as resp,
            tc.tile_pool(name="stat", bufs=3 * NG_S) as statp,
            tc.tile_pool(name="work", bufs=2) as work,
            tc.tile_pool(name="acc", bufs=4, space="PSUM") as accp,
            tc.tile_pool(name="tpp", bufs=4, space="PSUM") as tpp,
            tc.tile_pool(name="dram", bufs=1, space="DRAM") as dram,
        ):
            # ---- persistent small tensors ----
            w_sb = persist.tile([P, C_OUT], f32)
            nc.scalar.dma_start(w_sb[:], wf.ap())
            w4_sb = persist.tile([C_IN, C_OUT], f32)
            nc.scalar.dma_start(w4_sb[:], w4.ap())
            bias_sb = persist.tile([C_OUT, 1], f32)
            nc.scalar.dma_start(bias_sb[:], bias_in.ap())
            id_sb = persist.tile([P, C_IN], f32)
            nc.scalar.dma_start(id_sb[:], id128.ap())

            # T_0..T_3 stacked at partition bases {0,32,64,96} of one tile
            # (32-aligned so every engine may address them); T_4 separate.
            t_blk = persist.tile([P, VLOC], f32)
            t4_sb = persist.tile([C_IN, VLOC], f32)
            nc.scalar.dma_start(t_blk[0:C_IN, :], xc.ap())

            def t_ap(k):
                if k == K_CHEB - 1:
                    return t4_sb[:]
                return t_blk[32 * k:32 * k + C_IN, :]

            # stationary bands: band g holds T^T rows [g*1536, (g+1)*1536)
            # in plain j*128+p order; split A (j<8) / B (j>=8) so step k+1
            # matmuls on A-tiles need only the first AllGather half.
            xt_r = xt.ap().rearrange("(g p j) c -> g p j c", p=P, j=NB_S)
            sk = []
            for g in range(NG_S):
                a = statp.tile([P, NB_S * C_IN], mm_dt, name=f"st0_{g}",
                               tag="stat")
                nc.scalar.dma_start(
                    a[:].rearrange("p (j c) -> p j c", j=NB_S), xt_r[g])
                sk.append(a)

            def st_ap(j):
                g, jj = j // NB_S, j % NB_S
                return sk[g][:, jj * C_IN:(jj + 1) * C_IN]

            # tiny warm-up AllGathers: pay the first-collective cost
            # during step 1's DMA-bound phase, off the critical path
            wu_sb = work.tile([P, C_IN], mm_dt, name="wu_sb", tag="wu")
            nc.vector.memset(wu_sb[:], 0.0)
            for w in range(2):
                wu_in = dram.tile([P, C_IN], mm_dt, name=f"wu_in{w}")
                wu_out = dram.tile([P * N_CORES, C_IN], mm_dt,
                                   name=f"wu_out{w}")
                nc.scalar.dma_start(wu_in[:], wu_sb[:])
                nc.gpsimd.collective_compute(
                    "AllGather",
                    mybir.AluOpType.bypass,
                    replica_groups=[list(range(N_CORES))],
                    ins=[wu_in.opt()],
                    outs=[wu_out.opt()],
                )

            res_tiles = [None] * NR

            def lt_src(g, k):
                """SBUF tile + DMA for lt group g (0..NG-1) in step k."""
                if g >= NS:
                    ri = g - NS
                    if res_tiles[ri] is None:
                        t = resp.tile([P, NB * VLOC], mm_dt, name=f"res{ri}")
                        nc.sync.dma_start(
                            t[:].rearrange("p (u c) -> p u c", u=NB),
                            lt_r[g])
                        res_tiles[ri] = t
                    return res_tiles[ri]
                t = ltp.tile([P, NB * VLOC], mm_dt, name=f"lt{k}_{g}",
                             tag="lt")
                nc.sync.dma_start(
                    t[:].rearrange("p (u c) -> p u c", u=NB), lt_r[g])
                return t

            def mm_group(src, g, ch_list, acc, started, g_last):
                for u in range(NB):
                    j = g * NB + u
                    for ch in ch_list:
                        nc.tensor.matmul(
                            acc[ch][:],
                            lhsT=st_ap(j),
                            rhs=src[:, u * VLOC + ch * 512:
                                    u * VLOC + (ch + 1) * 512],
                            start=(ch not in started),
                            stop=(g == g_last and u == NB - 1),
                        )
                        started.add(ch)

            def drain(k, ch_list, acc):
                """T_k chunks -> transpose -> stage for AllGather half."""
                for ch in ch_list:
                    sl = slice(ch * 512, (ch + 1) * 512)
                    if k == 1:
                        nc.vector.tensor_copy(t_ap(k)[:, sl], acc[ch][:])
                    else:
                        nc.vector.scalar_tensor_tensor(
                            t_ap(k)[:, sl], acc[ch][:], 2.0,
                            t_ap(k - 2)[:, sl],
                            mybir.AluOpType.mult, mybir.AluOpType.subtract)

            # out[o, v] = sum_k w_k^T @ T_k + bias, one 512-col chunk.
            # T_0..T_3 ride the four PE row-groups concurrently, each into
            # its own PSUM bank (concurrent row-group drains must not
            # share a bank); T_4 accumulates serially onto row-group 0's.
            def einsum(ch):
                sl = slice(ch * 512, (ch + 1) * 512)
                ein = [accp.tile([C_OUT, 512], f32, name=f"ein{ch}_{kk}",
                                 tag="acc") for kk in range(K_CHEB - 1)]
                for kk in range(K_CHEB - 1):
                    tb = 32 * kk
                    nc.tensor.matmul(
                        ein[kk][:],
                        lhsT=w_sb[tb:tb + C_IN, :],
                        rhs=t_blk[tb:tb + C_IN, sl],
                        start=True,
                        stop=(kk != 0),
                        tile_position=(tb, 0) if tb == 96 else None,
                    )
                nc.tensor.matmul(ein[0][:], lhsT=w4_sb[:], rhs=t4_sb[:, sl],
                                 start=False, stop=True)
                # DVE reads at most one PSUM operand per op: chain the
                # reduction PSUM + SBUF -> SBUF.
                s0 = work.tile([C_OUT, 512], f32, name=f"s0_{ch}", tag="s0")
                nc.vector.tensor_scalar_add(s0[:], ein[0][:], bias_sb[:])
                s1 = work.tile([C_OUT, 512], f32, name=f"s1_{ch}", tag="s1")
                nc.vector.tensor_add(s1[:], ein[1][:], s0[:])
                s2 = work.tile([C_OUT, 512], f32, name=f"s2_{ch}", tag="s2")
                nc.vector.tensor_add(s2[:], ein[2][:], s1[:])
                res = work.tile([C_OUT, 512], f32, name=f"res{ch}",
                                tag="res")
                nc.vector.tensor_add(res[:], ein[3][:], s2[:])
                nc.scalar.dma_start(out.ap()[:, sl], res[:])

            for k in range(1, K_CHEB):
                acc = [accp.tile([C_IN, 512], f32, name=f"acc{k}_{ch}",
                                 tag="acc") for ch in range(N_CH)]
                started = set()
                # main phase: Bresenham-interleave streamed/resident so
                # streamed-group consumption stays below DMA delivery rate
                # across the whole step; step 1 is DMA-bound anyway so it
                # just streams in order.
                if k == 1:
                    main = list(range(NS)) + \
                        [NS + r for r in range(NR - TAIL)]
                else:
                    main = []
                    si, ri = 0, 0
                    nmain = NS + NR - TAIL
                    for i in range(nmain):
                        if si * (nmain) <= i * NS and si < NS:
                            main.append(si)
                            si += 1
                        else:
                            main.append(NS + ri)
                            ri += 1
                tail = [NS + r for r in range(NR - TAIL, NR)]

                for gi, g in enumerate(main):
                    src = lt_src(g, k)
                    mm_group(src, g, range(N_CH), acc, started, tail[-1])
                    if k == 1 and g == NS // 2:
                        # third warm-up AG pinned mid-step-1, size-matched
                        # to the real first AG half
                        wu_sb2 = work.tile([P, NB_S * C_IN], mm_dt,
                                           name="wu_sb2", tag="scs")
                        nc.vector.tensor_copy(wu_sb2[:],
                                              src[:, :NB_S * C_IN])
                        wu_in2 = dram.tile([NB_S * P, C_IN], mm_dt,
                                           name="wu_in2")
                        wu_out2 = dram.tile([NB_S * P * N_CORES, C_IN],
                                            mm_dt, name="wu_out2")
                        nc.scalar.dma_start(
                            wu_in2.rearrange("(p j) c -> p j c", p=P),
                            wu_sb2[:].rearrange("p (j c) -> p j c", j=NB_S))
                        nc.gpsimd.collective_compute(
                            "AllGather",
                            mybir.AluOpType.bypass,
                            replica_groups=[list(range(N_CORES))],
                            ins=[wu_in2.opt()],
                            outs=[wu_out2.opt()],
                        )

                # tail: last TAIL resident groups run chunks {0,1}
                # first, so T_k[:, :1024] drains while chunk-2 matmuls
                # still occupy the PE (overlaps einsum in step 4).
                for g in tail:
                    mm_group(lt_src(g, k), g, range(N_CH - 1), acc,
                             started, tail[-1])
                drain(k, range(N_CH - 1), acc)

                last = k == K_CHEB - 1
                if last:
                    for ch in range(N_CH - 1):
                        einsum(ch)
                for g in tail:
                    mm_group(lt_src(g, k), g, [N_CH - 1], acc, started,
                             tail[-1])
                drain(k, [N_CH - 1], acc)
                if last:
                    einsum(N_CH - 1)
                    continue

                # transpose T_k^T and all-gather it for the next step's
                # stationary bands (one collective; latency is fixed-cost
                # and skew dominated, so splitting it does not pay).
                sc_stage = work.tile([P, NB_S * C_IN], mm_dt,
                                     name=f"scs{k}", tag="scs")
                tb = 32 * k
                for j2 in range(NB_S):
                    tp_ps = tpp.tile([P, C_IN], f32, name=f"tp{k}_{j2}",
                                     tag="tp")
                    nc.tensor.transpose(
                        tp_ps[:],
                        t_blk[tb:tb + C_IN, j2 * P:(j2 + 1) * P],
                        id_sb[tb:tb + C_IN, :],
                        tile_position=(tb, 0) if tb == 96 else None)
                    nc.vector.tensor_copy(
                        sc_stage[:, j2 * C_IN:(j2 + 1) * C_IN], tp_ps[:])
                cc_in = dram.tile([VLOC, C_IN], mm_dt, name=f"ccin{k}")
                cc_out = dram.tile([V, C_IN], mm_dt, name=f"ccout{k}")
                nc.scalar.dma_start(
                    cc_in.rearrange("(p j) c -> p j c", p=P),
                    sc_stage[:].rearrange("p (j c) -> p j c", j=NB_S))
                nc.gpsimd.collective_compute(
                    "AllGather",
                    mybir.AluOpType.bypass,
                    replica_groups=[list(range(N_CORES))],
                    ins=[cc_in.opt()],
                    outs=[cc_out.opt()],
                )
                cc_r = cc_out.rearrange("(g p j) c -> g p j c",
                                        p=P, j=NB_S)
                sk = []
                for g in range(NG_S):
                    a = statp.tile([P, NB_S * C_IN], mm_dt,
                                   name=f"st{k}_{g}", tag="stat")
                    nc.scalar.dma_start(
                        a[:].rearrange("p (j c) -> p j c", j=NB_S),
                        cc_r[g])
                    sk.append(a)

    nc.compile()
    return nc


def _interleave_rows(a, nb):
    """Within each nb*128-row group, reorder rows so row g*G+nb*p+u holds
    original row g*G+u*128+p (one contiguous per-partition read)."""
    ng = a.shape[0] // (P * nb)
    return np.ascontiguousarray(
        a.reshape(ng, nb, P, a.shape[1]).transpose(0, 2, 1, 3)
        .reshape(a.shape))


def _prep_inputs(x, L, weights, bias, cfg: str):
    np_dt = ml_dtypes.bfloat16
    x = np.asarray(x, dtype=np.float32)
    L = np.asarray(L, dtype=np.float32)
    weights = np.asarray(weights, dtype=np.float32)
    bias = np.asarray(bias, dtype=np.float32)

    Lt = np.ascontiguousarray(L.T).astype(np_dt)          # (V, V)
    xt = _interleave_rows(
        np.ascontiguousarray(x.T).astype(np_dt), NB_S)    # (V, C_IN)

    wf = np.zeros((P, C_OUT), dtype=np.float32)
    for k in range(K_CHEB - 1):
        wf[32 * k:32 * k + C_IN, :] = weights[k]
    w4 = np.ascontiguousarray(weights[K_CHEB - 1])
    b_ = np.ascontiguousarray(bias.reshape(C_OUT, 1))
    id128 = np.zeros((P, C_IN), dtype=np.float32)
    for p in range(P):
        if p % 32 < C_IN:
            id128[p, p % 32] = 1.0

    in_maps = []
    for d in range(N_CORES):
        cols = slice(d * VLOC, (d + 1) * VLOC)
        in_maps.append({
            "lt": _interleave_rows(np.ascontiguousarray(Lt[:, cols]), NB),
            "xt": xt,
            "xc": np.ascontiguousarray(x[:, cols]),
            "wf": wf,
            "w4": w4,
            "bias_in": b_,
            "id128": id128,
        })
    return in_maps


def run(x, L, weights, bias, cfg: str = "bf16", trace: bool = False,
        trace_cores=None):
    if cfg not in _CACHE:
        _CACHE[cfg] = _build(cfg)
    nc = _CACHE[cfg]
    in_maps = _prep_inputs(x, L, weights, bias, cfg)
    kw = {}
    if trace_cores is not None:
        kw["trace_cores"] = trace_cores
    res = bass_utils.run_bass_kernel_spmd(
        nc, in_maps, core_ids=list(range(N_CORES)), trace=trace, **kw)
    out = np.concatenate([res.results[d]["out"] for d in range(N_CORES)],
                         axis=1)
    return out.astype(np.float32), res


def kernel(x, L, weights, bias):
    out, _ = run(x, L, weights, bias, cfg="bf16")
    return out


# revision 15
# speedup vs baseline: 1.1837x; 1.0152x over previous
import sys

if "/opt/trn_rl_repo" not in sys.path:
    sys.path.insert(0, "/opt/trn_rl_repo")

import numpy as np
import ml_dtypes

import concourse.bass as bass
import concourse.bacc as bacc
import concourse.tile as tile
import concourse.mybir as mybir
from concourse import bass_utils

# Problem shapes (nn_ChebConv): x (16, 12288), L (12288, 12288),
# weights (5, 16, 32), bias (32,). out (32, 12288).
#
# Sharding: core d owns V-columns [d*1536, (d+1)*1536).  Host feeds each
# core lt = L^T[:, cols_d] (contraction dim on partitions), row-interleaved
# within 256-row groups so each partition reads one contiguous chunk.
#
# Step k: psum(16,512)x3 accumulates T_{k-1} @ L^T over 96 vc-tiles.
# Half the lt slice (48 tiles) is resident in SBUF, loaded lazily during
# step 1 and reused DMA-free by steps 2-4; the other half re-streams each
# step.  Streamed and resident groups interleave so the PE is never
# DMA-paced; a resident-only tail at the end of each step is split by
# psum chunk so the transpose+AllGather of T_k[:, :1024] overlaps the
# chunk-2 matmuls, shrinking the step-boundary collective stall.
C_IN = 16
C_OUT = 32
K_CHEB = 5
V = 12288
N_CORES = 8
VLOC = V // N_CORES          # 1536 columns of the V axis per core
P = 128
NT_VC = V // P               # 96 contraction tiles per step
N_CH = VLOC // 512           # 3 psum chunks of 512
NB = 2                       # vc-tiles per lt DMA group (256 rows)
NG = NT_VC // NB             # 48 groups
NS = NG // 2                 # 24 streamed groups per step
NR = NG - NS                 # 24 resident groups
TAIL = 6                     # resident groups forming the chunk-split tail
LT_BUFS = 3
LT3_BUFS = 6
E3_STEPS = (1, 2, 3)         # steps whose streamed half reads e3m4 lt
E3_SCALE = 128.0             # global scale folded into stationary bands 0-3
H = V // 2                   # streamed rows of the contraction
NB_S = VLOC // P             # 12 vc-tiles per stationary band
NG_S = V // (P * NB_S)       # 8 stationary bands
JA = 8                       # vc-tiles of a band covered by AG half 1

_CACHE: dict = {}


def _build(cfg: str):
    mm_dt = mybir.dt.bfloat16
    f32 = mybir.dt.float32

    nc = bacc.Bacc("TRN2", target_bir_lowering=False, debug=False,
                   num_devices=N_CORES)

    lt = nc.dram_tensor("lt", [V, VLOC], mm_dt, kind="ExternalInput")
    lt3 = nc.dram_tensor("lt3", [H, VLOC], mybir.dt.float8e3,
                         kind="ExternalInput")
    xt = nc.dram_tensor("xt", [V, C_IN], mm_dt, kind="ExternalInput")
    xc = nc.dram_tensor("xc", [C_IN, VLOC], f32, kind="ExternalInput")
    wf = nc.dram_tensor("wf", [P, C_OUT], f32, kind="ExternalInput")
    w4 = nc.dram_tensor("w4", [C_IN, C_OUT], f32, kind="ExternalInput")
    bias_in = nc.dram_tensor("bias_in", [C_OUT, 1], f32, kind="ExternalInput")
    id128 = nc.dram_tensor("id128", [P, C_IN], f32, kind="ExternalInput")
    out = nc.dram_tensor("out", [C_OUT, VLOC], f32, kind="ExternalOutput")

    lt_r = lt.ap().rearrange("(g p u) c -> g p u c", p=P, u=NB)
    lt3_r = lt3.ap().rearrange("(g p u) c -> g p u c", p=P, u=NB)

    with tile.TileContext(nc) as tc:
        with (
            tc.tile_pool(name="ltp", bufs=LT_BUFS) as ltp,
            tc.tile_pool(name="ltp3", bufs=LT3_BUFS) as ltp3,
            tc.tile_pool(name="persist", bufs=1) as persist,
            tc.tile_pool(name="resp", bufs=1) as resp,
            tc.tile_pool(name="stat", bufs=2 * NG_S) as statp,
            tc.tile_pool(name="work", bufs=2) as work,
            tc.tile_pool(name="eins", bufs=1) as einsp,
            tc.tile_pool(name="acc", bufs=4, space="PSUM") as accp,
            tc.tile_pool(name="tpp", bufs=4, space="PSUM") as tpp,
            tc.tile_pool(name="dram", bufs=1, space="DRAM") as dram,
        ):
            # ---- persistent small tensors ----
            w_sb = persist.tile([P, C_OUT], f32)
            nc.scalar.dma_start(w_sb[:], wf.ap())
            w4_sb = persist.tile([C_IN, C_OUT], f32)
            nc.scalar.dma_start(w4_sb[:], w4.ap())
            bias_sb = persist.tile([C_OUT, 1], f32)
            nc.scalar.dma_start(bias_sb[:], bias_in.ap())
            id_sb = persist.tile([P, C_IN], f32)
            nc.scalar.dma_start(id_sb[:], id128.ap())

            # T_0..T_3 stacked at partition bases {0,32,64,96} of one tile
            # (32-aligned so every engine may address them); T_4 separate.
            t_blk = persist.tile([P, VLOC], f32)
            t4_sb = persist.tile([C_IN, VLOC], f32)
            nc.scalar.dma_start(t_blk[0:C_IN, :], xc.ap())

            def t_ap(k):
                if k == K_CHEB - 1:
                    return t4_sb[:]
                return t_blk[32 * k:32 * k + C_IN, :]

            # stationary bands: band g holds T^T rows [g*1536, (g+1)*1536)
            # in plain j*128+p order; split A (j<8) / B (j>=8) so step k+1
            # matmuls on A-tiles need only the first AllGather half.
            xt_r = xt.ap().rearrange("(g p j) c -> g p j c", p=P, j=NB_S)
            sk = []
            for g in range(NG_S):
                a = statp.tile([P, NB_S * C_IN], mm_dt, name=f"st0_{g}",
                               tag="stat")
                nc.scalar.dma_start(
                    a[:].rearrange("p (j c) -> p j c", j=NB_S), xt_r[g])
                sk.append(a)

            def st_ap(j):
                g, jj = j // NB_S, j % NB_S
                return sk[g][:, jj * C_IN:(jj + 1) * C_IN]

            # tiny warm-up AllGathers: pay the first-collective cost
            # during step 1's DMA-bound phase, off the critical path
            wu_sb = work.tile([P, C_IN], mm_dt, name="wu_sb", tag="wu")
            nc.vector.memset(wu_sb[:], 0.0)
            for w in range(2):
                wu_in = dram.tile([P, C_IN], mm_dt, name=f"wu_in{w}")
                wu_out = dram.tile([P * N_CORES, C_IN], mm_dt,
                                   name=f"wu_out{w}")
                nc.scalar.dma_start(wu_in[:], wu_sb[:])
                nc.gpsimd.collective_compute(
                    "AllGather",
                    mybir.AluOpType.bypass,
                    replica_groups=[list(range(N_CORES))],
                    ins=[wu_in.opt()],
                    outs=[wu_out.opt()],
                )

            res_tiles = [None] * NR

            def lt_src(g, k):
                """SBUF tile + DMA for lt group g (0..NG-1) in step k."""
                if g >= NS:
                    ri = g - NS
                    if res_tiles[ri] is None:
                        t = resp.tile([P, NB * VLOC], mm_dt, name=f"res{ri}")
                        nc.sync.dma_start(
                            t[:].rearrange("p (u c) -> p u c", u=NB),
                            lt_r[g])
                        res_tiles[ri] = t
                    return res_tiles[ri]
                if k in E3_STEPS:
                    t = ltp3.tile([P, NB * VLOC], mybir.dt.float8e3,
                                  name=f"lt3_{k}_{g}", tag="lt3")
                    nc.sync.dma_start(
                        t[:].rearrange("p (u c) -> p u c", u=NB), lt3_r[g])
                    return t
                t = ltp.tile([P, NB * VLOC], mm_dt, name=f"lt{k}_{g}",
                             tag="lt")
                nc.sync.dma_start(
                    t[:].rearrange("p (u c) -> p u c", u=NB), lt_r[g])
                return t

            def mm_group(src, g, ch_list, acc, started, g_last):
                for u in range(NB):
                    j = g * NB + u
                    for ch in ch_list:
                        nc.tensor.matmul(
                            acc[ch][:],
                            lhsT=st_ap(j),
                            rhs=src[:, u * VLOC + ch * 512:
                                    u * VLOC + (ch + 1) * 512],
                            start=(ch not in started),
                            stop=(g == g_last and u == NB - 1),
                        )
                        started.add(ch)

            def drain(k, ch_list, acc):
                """T_k chunks -> transpose -> stage for AllGather half."""
                for ch in ch_list:
                    sl = slice(ch * 512, (ch + 1) * 512)
                    if k == 1:
                        nc.vector.tensor_copy(t_ap(k)[:, sl], acc[ch][:])
                    else:
                        nc.vector.scalar_tensor_tensor(
                            t_ap(k)[:, sl], acc[ch][:], 2.0,
                            t_ap(k - 2)[:, sl],
                            mybir.AluOpType.mult, mybir.AluOpType.subtract)

            # out[o, v] = sum_k w_k^T @ T_k + bias, one 512-col chunk.
            # T_0..T_3 ride the four PE row-groups concurrently, each into
            # its own PSUM bank (concurrent row-group drains must not
            # share a bank); T_4 accumulates serially onto row-group 0's.
            def einsum(ch):
                sl = slice(ch * 512, (ch + 1) * 512)
                ein = [accp.tile([C_OUT, 512], f32, name=f"ein{ch}_{kk}",
                                 tag="acc") for kk in range(K_CHEB - 1)]
                for kk in range(K_CHEB - 1):
                    tb = 32 * kk
                    nc.tensor.matmul(
                        ein[kk][:],
                        lhsT=w_sb[tb:tb + C_IN, :],
                        rhs=t_blk[tb:tb + C_IN, sl],
                        start=True,
                        stop=(kk != 0),
                        tile_position=(tb, 0) if tb == 96 else None,
                    )
                nc.tensor.matmul(ein[0][:], lhsT=w4_sb[:], rhs=t4_sb[:, sl],
                                 start=False, stop=True)
                # DVE reads at most one PSUM operand per op: chain the
                # reduction PSUM + SBUF -> SBUF.
                s0 = einsp.tile([C_OUT, 512], f32, name=f"s0_{ch}", tag="s0")
                nc.vector.tensor_scalar_add(s0[:], ein[0][:], bias_sb[:])
                s1 = einsp.tile([C_OUT, 512], f32, name=f"s1_{ch}", tag="s1")
                nc.vector.tensor_add(s1[:], ein[1][:], s0[:])
                s2 = einsp.tile([C_OUT, 512], f32, name=f"s2_{ch}", tag="s2")
                nc.vector.tensor_add(s2[:], ein[2][:], s1[:])
                res = einsp.tile([C_OUT, 512], f32, name=f"res{ch}",
                                tag="res")
                nc.vector.tensor_add(res[:], ein[3][:], s2[:])
                nc.scalar.dma_start(out.ap()[:, sl], res[:])

            for k in range(1, K_CHEB):
                acc = [accp.tile([C_IN, 512], f32, name=f"acc{k}_{ch}",
                                 tag="acc") for ch in range(N_CH)]
                started = set()
                # main phase: Bresenham-interleave streamed/resident so
                # streamed-group consumption stays below DMA delivery rate
                # across the whole step; step 1 is DMA-bound anyway so it
                # just streams in order.
                if k == 1:
                    main = list(range(NS)) + \
                        [NS + r for r in range(NR - TAIL)]
                else:
                    main = []
                    si, ri = 0, 0
                    nmain = NS + NR - TAIL
                    for i in range(nmain):
                        if si * (nmain) <= i * NS and si < NS:
                            main.append(si)
                            si += 1
                        else:
                            main.append(NS + ri)
                            ri += 1
                tail = [NS + r for r in range(NR - TAIL, NR)]

                for gi, g in enumerate(main):
                    src = lt_src(g, k)
                    mm_group(src, g, range(N_CH), acc, started, tail[-1])
                    if k == 1 and g == NS // 2:
                        # third warm-up AG pinned mid-step-1, size-matched
                        # to the real first AG half
                        wu_sb2 = work.tile([P, NB_S * C_IN], mm_dt,
                                           name="wu_sb2", tag="scs")
                        nc.vector.tensor_copy(wu_sb2[:],
                                              src[:, :NB_S * C_IN])
                        wu_in2 = dram.tile([NB_S * P, C_IN], mm_dt,
                                           name="wu_in2")
                        wu_out2 = dram.tile([NB_S * P * N_CORES, C_IN],
                                            mm_dt, name="wu_out2")
                        nc.scalar.dma_start(
                            wu_in2.rearrange("(p j) c -> p j c", p=P),
                            wu_sb2[:].rearrange("p (j c) -> p j c", j=NB_S))
                        nc.gpsimd.collective_compute(
                            "AllGather",
                            mybir.AluOpType.bypass,
                            replica_groups=[list(range(N_CORES))],
                            ins=[wu_in2.opt()],
                            outs=[wu_out2.opt()],
                        )

                # tail: last TAIL resident groups run chunks {0,1}
                # first, so T_k[:, :1024] drains while chunk-2 matmuls
                # still occupy the PE (overlaps einsum in step 4).
                for g in tail:
                    mm_group(lt_src(g, k), g, range(N_CH - 1), acc,
                             started, tail[-1])
                drain(k, range(N_CH - 1), acc)

                last = k == K_CHEB - 1
                if last:
                    for ch in range(N_CH - 1):
                        einsum(ch)
                for g in tail:
                    mm_group(lt_src(g, k), g, [N_CH - 1], acc, started,
                             tail[-1])
                drain(k, [N_CH - 1], acc)
                if last:
                    einsum(N_CH - 1)
                    continue

                # transpose T_k^T and all-gather it for the next step's
                # stationary bands (one collective; latency is fixed-cost
                # and skew dominated, so splitting it does not pay).
                sc_stage = work.tile([P, NB_S * C_IN], mm_dt,
                                     name=f"scs{k}", tag="scs")
                tb = 32 * k
                for j2 in range(NB_S):
                    tp_ps = tpp.tile([P, C_IN], f32, name=f"tp{k}_{j2}",
                                     tag="tp")
                    nc.tensor.transpose(
                        tp_ps[:],
                        t_blk[tb:tb + C_IN, j2 * P:(j2 + 1) * P],
                        id_sb[tb:tb + C_IN, :],
                        tile_position=(tb, 0) if tb == 96 else None)
                    nc.vector.tensor_copy(
                        sc_stage[:, j2 * C_IN:(j2 + 1) * C_IN], tp_ps[:])
                cc_in = dram.tile([VLOC, C_IN], mm_dt, name=f"ccin{k}")
                cc_out = dram.tile([V, C_IN], mm_dt, name=f"ccout{k}")
                nc.scalar.dma_start(
                    cc_in.rearrange("(p j) c -> p j c", p=P),
                    sc_stage[:].rearrange("p (j c) -> p j c", j=NB_S))
                nc.gpsimd.collective_compute(
                    "AllGather",
                    mybir.AluOpType.bypass,
                    replica_groups=[list(range(N_CORES))],
                    ins=[cc_in.opt()],
                    outs=[cc_out.opt()],
                )
                cc_r = cc_out.rearrange("(g p j) c -> g p j c",
                                        p=P, j=NB_S)
                sk = []
                nxt_e3 = (k + 1) in E3_STEPS
                for g in range(NG_S):
                    a = statp.tile([P, NB_S * C_IN], mm_dt,
                                   name=f"st{k}_{g}", tag="stat")
                    eng = nc.scalar if g % 2 == 0 else nc.sync
                    eng.dma_start(
                        a[:].rearrange("p (j c) -> p j c", j=NB_S),
                        cc_r[g])
                    if nxt_e3 and g < NG_S // 2:
                        nc.vector.tensor_scalar_mul(
                            a[:], a[:], 1.0 / E3_SCALE)
                    sk.append(a)

    nc.compile()
    return nc


def _interleave_rows(a, nb):
    """Within each nb*128-row group, reorder rows so row g*G+nb*p+u holds
    original row g*G+u*128+p (one contiguous per-partition read)."""
    ng = a.shape[0] // (P * nb)
    return np.ascontiguousarray(
        a.reshape(ng, nb, P, a.shape[1]).transpose(0, 2, 1, 3)
        .reshape(a.shape))


def _prep_inputs(x, L, weights, bias, cfg: str):
    np_dt = ml_dtypes.bfloat16
    x = np.asarray(x, dtype=np.float32)
    L = np.asarray(L, dtype=np.float32)
    weights = np.asarray(weights, dtype=np.float32)
    bias = np.asarray(bias, dtype=np.float32)

    Lt = np.ascontiguousarray(L.T)                        # (V, V) f32
    Lt3 = (Lt[:H] * E3_SCALE).astype(ml_dtypes.float8_e3m4)
    Lt = Lt.astype(np_dt)
    xtf = np.ascontiguousarray(x.T).astype(np.float32)    # (V, C_IN)
    if 1 in E3_STEPS:
        xtf[:H] /= E3_SCALE       # fold 1/scale into stationary bands 0-3
    xt = _interleave_rows(xtf.astype(np_dt), NB_S)

    wf = np.zeros((P, C_OUT), dtype=np.float32)
    for k in range(K_CHEB - 1):
        wf[32 * k:32 * k + C_IN, :] = weights[k]
    w4 = np.ascontiguousarray(weights[K_CHEB - 1])
    b_ = np.ascontiguousarray(bias.reshape(C_OUT, 1))
    id128 = np.zeros((P, C_IN), dtype=np.float32)
    for p in range(P):
        if p % 32 < C_IN:
            id128[p, p % 32] = 1.0

    in_maps = []
    for d in range(N_CORES):
        cols = slice(d * VLOC, (d + 1) * VLOC)
        in_maps.append({
            "lt": _interleave_rows(np.ascontiguousarray(Lt[:, cols]), NB),
            "lt3": _interleave_rows(np.ascontiguousarray(Lt3[:, cols]), NB),
            "xt": xt,
            "xc": np.ascontiguousarray(x[:, cols]),
            "wf": wf,
            "w4": w4,
            "bias_in": b_,
            "id128": id128,
        })
    return in_maps


def run(x, L, weights, bias, cfg: str = "bf16", trace: bool = False,
        trace_cores=None):
    if cfg not in _CACHE:
        _CACHE[cfg] = _build(cfg)
    nc = _CACHE[cfg]
    in_maps = _prep_inputs(x, L, weights, bias, cfg)
    kw = {}
    if trace_cores is not None:
        kw["trace_cores"] = trace_cores
    res = bass_utils.run_bass_kernel_spmd(
        nc, in_maps, core_ids=list(range(N_CORES)), trace=trace, **kw)
    out = np.concatenate([res.results[d]["out"] for d in range(N_CORES)],
                         axis=1)
    return out.astype(np.float32), res


def kernel(x, L, weights, bias):
    out, _ = run(x, L, weights, bias, cfg="bf16")
    return out


# revision 16
# speedup vs baseline: 1.2443x; 1.0511x over previous
import sys

if "/opt/trn_rl_repo" not in sys.path:
    sys.path.insert(0, "/opt/trn_rl_repo")

import numpy as np
import ml_dtypes

import concourse.bass as bass
import concourse.bacc as bacc
import concourse.tile as tile
import concourse.mybir as mybir
from concourse import bass_utils

# Problem shapes (nn_ChebConv): x (16, 12288), L (12288, 12288),
# weights (5, 16, 32), bias (32,). out (32, 12288).
#
# Sharding: core d owns V-columns [d*1536, (d+1)*1536).  Host feeds each
# core lt = L^T[:, cols_d] (contraction dim on partitions), row-interleaved
# within 256-row groups so each partition reads one contiguous chunk.
#
# Step k: psum(16,512)x3 accumulates T_{k-1} @ L^T over 96 vc-tiles.
# Half the lt slice (48 tiles) is resident in SBUF, loaded lazily during
# step 1 and reused DMA-free by steps 2-4; the other half re-streams each
# step.  Streamed and resident groups interleave so the PE is never
# DMA-paced; a resident-only tail at the end of each step is split by
# psum chunk so the transpose+AllGather of T_k[:, :1024] overlaps the
# chunk-2 matmuls, shrinking the step-boundary collective stall.
C_IN = 16
C_OUT = 32
K_CHEB = 5
V = 12288
N_CORES = 8
VLOC = V // N_CORES          # 1536 columns of the V axis per core
P = 128
NT_VC = V // P               # 96 contraction tiles per step
N_CH = VLOC // 512           # 3 psum chunks of 512
NB = 2                       # vc-tiles per lt DMA group (256 rows)
NG = NT_VC // NB             # 48 groups
NS = NG // 2                 # 24 streamed groups per step
NR = NG - NS                 # 24 resident groups
TAIL = 6                     # resident groups forming the chunk-split tail
LT3_BUFS = 8
E3_STEPS = (1, 2, 3, 4)      # steps whose streamed half reads e3m4 lt
E3_SCALE = 128.0             # global scale folded into stationary bands 0-3
H = V // 2                   # streamed rows of the contraction
NB_S = VLOC // P             # 12 vc-tiles per stationary band
NG_S = V // (P * NB_S)       # 8 stationary bands
JA = 8                       # vc-tiles of a band covered by AG half 1

_CACHE: dict = {}


def _build(cfg: str):
    mm_dt = mybir.dt.bfloat16
    f32 = mybir.dt.float32

    nc = bacc.Bacc("TRN2", target_bir_lowering=False, debug=False,
                   num_devices=N_CORES)

    lt = nc.dram_tensor("lt", [V, VLOC], mm_dt, kind="ExternalInput")
    lt3 = nc.dram_tensor("lt3", [H, VLOC], mybir.dt.float8e3,
                         kind="ExternalInput")
    xt = nc.dram_tensor("xt", [V, C_IN], mm_dt, kind="ExternalInput")
    xc = nc.dram_tensor("xc", [C_IN, VLOC], f32, kind="ExternalInput")
    wf = nc.dram_tensor("wf", [P, C_OUT], f32, kind="ExternalInput")
    w4 = nc.dram_tensor("w4", [C_IN, C_OUT], f32, kind="ExternalInput")
    bias_in = nc.dram_tensor("bias_in", [C_OUT, 1], f32, kind="ExternalInput")
    id128 = nc.dram_tensor("id128", [P, C_IN], f32, kind="ExternalInput")
    out = nc.dram_tensor("out", [C_OUT, VLOC], f32, kind="ExternalOutput")

    lt_r = lt.ap().rearrange("(g p u) c -> g p u c", p=P, u=NB)
    lt3_r = lt3.ap().rearrange("(g p u) c -> g p u c", p=P, u=NB)

    with tile.TileContext(nc) as tc:
        with (
            tc.tile_pool(name="ltp3", bufs=LT3_BUFS) as ltp3,
            tc.tile_pool(name="persist", bufs=1) as persist,
            tc.tile_pool(name="resp", bufs=1) as resp,
            tc.tile_pool(name="stat", bufs=2 * NG_S) as statp,
            tc.tile_pool(name="work", bufs=2) as work,
            tc.tile_pool(name="eins", bufs=1) as einsp,
            tc.tile_pool(name="acc", bufs=4, space="PSUM") as accp,
            tc.tile_pool(name="tpp", bufs=4, space="PSUM") as tpp,
            tc.tile_pool(name="dram", bufs=1, space="DRAM") as dram,
        ):
            # warm-up AllGathers first: the first collective costs ~60us
            # of one-time CC init after its trigger, and each small AG is
            # ~16us; trigger both at t=0 so the chain drains inside step 1
            # and the real AG1 finds an idle CC queue.
            wu_sb = work.tile([P, NB_S * C_IN], mm_dt, name="wu_sb",
                              tag="wu")
            nc.vector.memset(wu_sb[:], 0.0)
            for w, jw in ((0, 1), (1, NB_S)):
                wu_in = dram.tile([jw * P, C_IN], mm_dt, name=f"wu_in{w}")
                wu_out = dram.tile([jw * P * N_CORES, C_IN], mm_dt,
                                   name=f"wu_out{w}")
                nc.scalar.dma_start(
                    wu_in.rearrange("(p j) c -> p j c", p=P),
                    wu_sb[:, :jw * C_IN]
                    .rearrange("p (j c) -> p j c", j=jw))
                nc.gpsimd.collective_compute(
                    "AllGather",
                    mybir.AluOpType.bypass,
                    replica_groups=[list(range(N_CORES))],
                    ins=[wu_in.opt()],
                    outs=[wu_out.opt()],
                )

            # ---- persistent small tensors ----
            w_sb = persist.tile([P, C_OUT], f32)
            nc.scalar.dma_start(w_sb[:], wf.ap())
            w4_sb = persist.tile([C_IN, C_OUT], f32)
            nc.scalar.dma_start(w4_sb[:], w4.ap())
            bias_sb = persist.tile([C_OUT, 1], f32)
            nc.scalar.dma_start(bias_sb[:], bias_in.ap())
            id_sb = persist.tile([P, C_IN], f32)
            nc.scalar.dma_start(id_sb[:], id128.ap())

            # T_0..T_3 stacked at partition bases {0,32,64,96} of one tile
            # (32-aligned so every engine may address them); T_4 separate.
            t_blk = persist.tile([P, VLOC], f32)
            t4_sb = persist.tile([C_IN, VLOC], f32)
            nc.scalar.dma_start(t_blk[0:C_IN, :], xc.ap())

            def t_ap(k):
                if k == K_CHEB - 1:
                    return t4_sb[:]
                return t_blk[32 * k:32 * k + C_IN, :]

            # stationary bands: band g holds T^T rows [g*1536, (g+1)*1536)
            # in plain j*128+p order; split A (j<8) / B (j>=8) so step k+1
            # matmuls on A-tiles need only the first AllGather half.
            xt_r = xt.ap().rearrange("(g p j) c -> g p j c", p=P, j=NB_S)
            sk = []
            for g in range(NG_S):
                a = statp.tile([P, NB_S * C_IN], mm_dt, name=f"st0_{g}",
                               tag="stat")
                nc.scalar.dma_start(
                    a[:].rearrange("p (j c) -> p j c", j=NB_S), xt_r[g])
                sk.append(a)

            def st_ap(j):
                g, jj = j // NB_S, j % NB_S
                return sk[g][:, jj * C_IN:(jj + 1) * C_IN]

            res_tiles = [None] * NR

            def lt_src(g, k):
                """SBUF tile + DMA for lt group g (0..NG-1) in step k."""
                if g >= NS:
                    ri = g - NS
                    if res_tiles[ri] is None:
                        t = resp.tile([P, NB * VLOC], mm_dt, name=f"res{ri}")
                        nc.sync.dma_start(
                            t[:].rearrange("p (u c) -> p u c", u=NB),
                            lt_r[g])
                        res_tiles[ri] = t
                    return res_tiles[ri]
                t = ltp3.tile([P, NB * VLOC], mybir.dt.float8e3,
                              name=f"lt3_{k}_{g}", tag="lt3")
                nc.sync.dma_start(
                    t[:].rearrange("p (u c) -> p u c", u=NB), lt3_r[g])
                return t

            def mm_group(src, g, ch_list, acc, started, g_last):
                for u in range(NB):
                    j = g * NB + u
                    for ch in ch_list:
                        nc.tensor.matmul(
                            acc[ch][:],
                            lhsT=st_ap(j),
                            rhs=src[:, u * VLOC + ch * 512:
                                    u * VLOC + (ch + 1) * 512],
                            start=(ch not in started),
                            stop=(g == g_last and u == NB - 1),
                        )
                        started.add(ch)

            def drain(k, ch_list, acc):
                """T_k chunks -> transpose -> stage for AllGather half."""
                for ch in ch_list:
                    sl = slice(ch * 512, (ch + 1) * 512)
                    if k == 1:
                        nc.vector.tensor_copy(t_ap(k)[:, sl], acc[ch][:])
                    else:
                        nc.vector.scalar_tensor_tensor(
                            t_ap(k)[:, sl], acc[ch][:], 2.0,
                            t_ap(k - 2)[:, sl],
                            mybir.AluOpType.mult, mybir.AluOpType.subtract)

            # out[o, v] = sum_k w_k^T @ T_k + bias, one 512-col chunk.
            # T_0..T_3 ride the four PE row-groups concurrently, each into
            # its own PSUM bank (concurrent row-group drains must not
            # share a bank); T_4 accumulates serially onto row-group 0's.
            def einsum(ch):
                sl = slice(ch * 512, (ch + 1) * 512)
                ein = [accp.tile([C_OUT, 512], f32, name=f"ein{ch}_{kk}",
                                 tag="acc") for kk in range(K_CHEB - 1)]
                for kk in range(K_CHEB - 1):
                    tb = 32 * kk
                    nc.tensor.matmul(
                        ein[kk][:],
                        lhsT=w_sb[tb:tb + C_IN, :],
                        rhs=t_blk[tb:tb + C_IN, sl],
                        start=True,
                        stop=(kk != 0),
                        tile_position=(tb, 0) if tb == 96 else None,
                    )
                nc.tensor.matmul(ein[0][:], lhsT=w4_sb[:], rhs=t4_sb[:, sl],
                                 start=False, stop=True)
                # DVE reads at most one PSUM operand per op: chain the
                # reduction PSUM + SBUF -> SBUF.
                s0 = einsp.tile([C_OUT, 512], f32, name=f"s0_{ch}", tag="s0")
                nc.vector.tensor_scalar_add(s0[:], ein[0][:], bias_sb[:])
                s1 = einsp.tile([C_OUT, 512], f32, name=f"s1_{ch}", tag="s1")
                nc.vector.tensor_add(s1[:], ein[1][:], s0[:])
                s2 = einsp.tile([C_OUT, 512], f32, name=f"s2_{ch}", tag="s2")
                nc.vector.tensor_add(s2[:], ein[2][:], s1[:])
                res = einsp.tile([C_OUT, 512], f32, name=f"res{ch}",
                                tag="res")
                nc.vector.tensor_add(res[:], ein[3][:], s2[:])
                nc.scalar.dma_start(out.ap()[:, sl], res[:])

            for k in range(1, K_CHEB):
                acc = [accp.tile([C_IN, 512], f32, name=f"acc{k}_{ch}",
                                 tag="acc") for ch in range(N_CH)]
                started = set()
                # main phase: Bresenham-interleave streamed/resident so
                # streamed-group consumption stays below DMA delivery rate
                # across the whole step; step 1 is DMA-bound anyway so it
                # just streams in order.
                if k == 1:
                    main = list(range(NS)) + \
                        [NS + r for r in range(NR - TAIL)]
                else:
                    main = []
                    si, ri = 0, 0
                    nmain = NS + NR - TAIL
                    for i in range(nmain):
                        if si * (nmain) <= i * NS and si < NS:
                            main.append(si)
                            si += 1
                        else:
                            main.append(NS + ri)
                            ri += 1
                tail = [NS + r for r in range(NR - TAIL, NR)]

                for gi, g in enumerate(main):
                    src = lt_src(g, k)
                    mm_group(src, g, range(N_CH), acc, started, tail[-1])
                # tail: last TAIL resident groups run chunks {0,1}
                # first, so T_k[:, :1024] drains while chunk-2 matmuls
                # still occupy the PE (overlaps einsum in step 4).
                for g in tail:
                    mm_group(lt_src(g, k), g, range(N_CH - 1), acc,
                             started, tail[-1])
                drain(k, range(N_CH - 1), acc)

                last = k == K_CHEB - 1
                if last:
                    for ch in range(N_CH - 1):
                        einsum(ch)
                for g in tail:
                    mm_group(lt_src(g, k), g, [N_CH - 1], acc, started,
                             tail[-1])
                drain(k, [N_CH - 1], acc)
                if last:
                    einsum(N_CH - 1)
                    continue

                # transpose T_k^T and all-gather it for the next step's
                # stationary bands (one collective; latency is fixed-cost
                # and skew dominated, so splitting it does not pay).
                sc_stage = work.tile([P, NB_S * C_IN], mm_dt,
                                     name=f"scs{k}", tag="scs")
                tb = 32 * k
                for j2 in range(NB_S):
                    tp_ps = tpp.tile([P, C_IN], f32, name=f"tp{k}_{j2}",
                                     tag="tp")
                    nc.tensor.transpose(
                        tp_ps[:],
                        t_blk[tb:tb + C_IN, j2 * P:(j2 + 1) * P],
                        id_sb[tb:tb + C_IN, :],
                        tile_position=(tb, 0) if tb == 96 else None)
                    nc.vector.tensor_copy(
                        sc_stage[:, j2 * C_IN:(j2 + 1) * C_IN], tp_ps[:])
                cc_in = dram.tile([VLOC, C_IN], mm_dt, name=f"ccin{k}")
                cc_out = dram.tile([V, C_IN], mm_dt, name=f"ccout{k}")
                nc.scalar.dma_start(
                    cc_in.rearrange("(p j) c -> p j c", p=P),
                    sc_stage[:].rearrange("p (j c) -> p j c", j=NB_S))
                nc.gpsimd.collective_compute(
                    "AllGather",
                    mybir.AluOpType.bypass,
                    replica_groups=[list(range(N_CORES))],
                    ins=[cc_in.opt()],
                    outs=[cc_out.opt()],
                )
                cc_r = cc_out.rearrange("(g p j) c -> g p j c",
                                        p=P, j=NB_S)
                sk = []
                nxt_e3 = (k + 1) in E3_STEPS
                for g in range(NG_S):
                    a = statp.tile([P, NB_S * C_IN], mm_dt,
                                   name=f"st{k}_{g}", tag="stat")
                    eng = nc.scalar if g % 2 == 0 else nc.sync
                    eng.dma_start(
                        a[:].rearrange("p (j c) -> p j c", j=NB_S),
                        cc_r[g])
                    if nxt_e3 and g < NG_S // 2:
                        nc.vector.tensor_scalar_mul(
                            a[:], a[:], 1.0 / E3_SCALE)
                    sk.append(a)

    nc.compile()
    return nc


def _interleave_rows(a, nb):
    """Within each nb*128-row group, reorder rows so row g*G+nb*p+u holds
    original row g*G+u*128+p (one contiguous per-partition read)."""
    ng = a.shape[0] // (P * nb)
    return np.ascontiguousarray(
        a.reshape(ng, nb, P, a.shape[1]).transpose(0, 2, 1, 3)
        .reshape(a.shape))


def _prep_inputs(x, L, weights, bias, cfg: str):
    np_dt = ml_dtypes.bfloat16
    x = np.asarray(x, dtype=np.float32)
    L = np.asarray(L, dtype=np.float32)
    weights = np.asarray(weights, dtype=np.float32)
    bias = np.asarray(bias, dtype=np.float32)

    Lt = np.ascontiguousarray(L.T)                        # (V, V) f32
    Lt3 = (Lt[:H] * E3_SCALE).astype(ml_dtypes.float8_e3m4)
    Lt = Lt.astype(np_dt)
    xtf = np.ascontiguousarray(x.T).astype(np.float32)    # (V, C_IN)
    if 1 in E3_STEPS:
        xtf[:H] /= E3_SCALE       # fold 1/scale into stationary bands 0-3
    xt = _interleave_rows(xtf.astype(np_dt), NB_S)

    wf = np.zeros((P, C_OUT), dtype=np.float32)
    for k in range(K_CHEB - 1):
        wf[32 * k:32 * k + C_IN, :] = weights[k]
    w4 = np.ascontiguousarray(weights[K_CHEB - 1])
    b_ = np.ascontiguousarray(bias.reshape(C_OUT, 1))
    id128 = np.zeros((P, C_IN), dtype=np.float32)
    for p in range(P):
        if p % 32 < C_IN:
            id128[p, p % 32] = 1.0

    in_maps = []
    for d in range(N_CORES):
        cols = slice(d * VLOC, (d + 1) * VLOC)
        in_maps.append({
            "lt": _interleave_rows(np.ascontiguousarray(Lt[:, cols]), NB),
            "lt3": _interleave_rows(np.ascontiguousarray(Lt3[:, cols]), NB),
            "xt": xt,
            "xc": np.ascontiguousarray(x[:, cols]),
            "wf": wf,
            "w4": w4,
            "bias_in": b_,
            "id128": id128,
        })
    return in_maps


def run(x, L, weights, bias, cfg: str = "bf16", trace: bool = False,
        trace_cores=None):
    if cfg not in _CACHE:
        _CACHE[cfg] = _build(cfg)
    nc = _CACHE[cfg]
    in_maps = _prep_inputs(x, L, weights, bias, cfg)
    kw = {}
    if trace_cores is not None:
        kw["trace_cores"] = trace_cores
    res = bass_utils.run_bass_kernel_spmd(
        nc, in_maps, core_ids=list(range(N_CORES)), trace=trace, **kw)
    out = np.concatenate([res.results[d]["out"] for d in range(N_CORES)],
                         axis=1)
    return out.astype(np.float32), res


def kernel(x, L, weights, bias):
    out, _ = run(x, L, weights, bias, cfg="bf16")
    return out
